# revision 21
# baseline (speedup 1.0000x reference)
"""Trainium2 Bass kernel for DigitConvolutionalModel (self-contained).

Model: out = relu(conv3x3(x) @ w1.T + b1) @ w2.T + b2, x: [65536, 784] f32.

The 3x3 valid cross-correlation is linear in x, so it is folded into the
first linear layer on the host, giving a 2-layer MLP:
out = relu(x @ W1_eff.T + b1) @ w2.T + b2.

Sharding: pure data parallelism - batch split 8 ways (8192 rows/core),
weights replicated; no collectives. Per core the kernel computes
out.T [10, 8192] with batch on the matmul free dim and features on
partitions. Host casts 2*x to fp8 E3M4 (scale 2 halves the subnormal
fraction; 1/2 folded into bf16 W1) and lays it out in blocked SBUF tile
order. Features are tiled 7 x 112 (784 exactly), so there is no K=16
remainder pass: per 1024-col chunk the PE runs 14 L1 matmuls + 2 L2
matmuls, all N=512 at ~216 ns (fp8e3 moving = 1 col/cycle @2.4GHz).

Schedule notes (from trace analysis):
 - PE p-state reaches 2.4 GHz only after ~8-9 us of busy time; warm-up
   matmuls run from block start and bridge into the first data with no
   idle gap (a gap resets progress and costs ~2x matmuls for a while).
 - DMA completion semaphores trail the data descriptors by 1.5-5 us when
   the queue is deep, so the stream uses half-chunk granularity and the
   PE never waits on a DMA issued less than ~2 chunks earlier; chunk 0
   streams as 4 small pieces (1+1+2+3 k-blocks) consumed k-by-k.
 - Tensor-queue semaphore waits are hoisted a couple of matmuls before
   the group boundary they guard so LDWEIGHTS prefetch is not blocked
   (kills ~100 ns per stationary switch).
 - Last chunk is processed s0[0:512] then s1[512:1024] (PSUM-bank
   aligned: the relu of one half may not touch the bank the PE is still
   writing), with relu/L2/copy/output pipelined into the PE tail; final
   output goes out as two half DMAs on the (idle by then) Sync queue.
 - hidden dim padded 100 -> 128 with zero weight columns; b1_pad[100]=1
   makes h1 row 100 == 1.0 and W2T row 100 = b2, folding the second-layer
   bias into the second matmul.
"""

import sys

import numpy as np

if "/opt/trn_rl_repo" not in sys.path:
    sys.path.insert(0, "/opt/trn_rl_repo")

import ml_dtypes

B = 65536
IMG = 28
KSZ = 3
OUT_HW = IMG - KSZ + 1  # 26
FLAT = OUT_HW * OUT_HW  # 676
HID = 100
NCLS = 10
FEAT = IMG * IMG  # 784

N_CORES = 8
BPC = B // N_CORES  # 8192 batch rows per core
KB = 7  # feature k-blocks
KP = FEAT // KB  # 112 features per block
KH0 = 4  # k-blocks in each chunk's first half DMA
HPAD = 128
NB = 1024  # batch rows per chunk
NCHUNK = BPC // NB  # 8
XBYTES = KB * NB  # 7168 bytes per partition per chunk

NXS = 8  # half-chunk x slot ring (chunks 1-7 -> 14 halves)
NPS1 = 3
NPS2 = 2
NH1 = 3
NOB = 3
NWARM = 7
CPK_BYTES = 1816
CPK_SPLIT = 512  # W1[k0..k1] in the first consts DMA, rest in the second
C0_PIECES = ((0, 1), (1, 2), (2, 4), (4, 7))  # chunk-0 k-block pieces

_BF16 = ml_dtypes.bfloat16
_F8 = ml_dtypes.float8_e3m4
XSCALE = 2.0  # x pre-scale before fp8 (1/XSCALE folded into W1)
_CACHE = {}


def _enable_ldw_opt():
    # Rebind concourse.bass_utils.bir_verify_and_optimise with walrus's
    # --enable-ldw-opt flipped on: consecutive matmuls sharing a stationary
    # tensor reuse the loaded weights. Falls back silently if the source no
    # longer matches.
    if _CACHE.get("ldw_patched"):
        return
    _CACHE["ldw_patched"] = True
    try:
        import inspect

        import concourse.bass_utils as bu

        src = inspect.getsource(bu.bir_verify_and_optimise)
        if "--enable-ldw-opt=false" in src:
            src = src.replace("--enable-ldw-opt=false", "--enable-ldw-opt=true")
            exec(compile(src, bu.__file__, "exec"), bu.__dict__)
    except Exception:
        pass


def _build_module():
    import contextlib

    from concourse import bacc, mybir

    _enable_ldw_opt()

    nc = bacc.Bacc(
        "TRN2", target_bir_lowering=False, debug=False, num_devices=N_CORES
    )
    xm = nc.dram_tensor(
        "xm", [NCHUNK, KP, XBYTES], mybir.dt.float8e3, kind="ExternalInput"
    ).ap()
    cpk = nc.dram_tensor(
        "cpk", [128, CPK_BYTES], mybir.dt.uint8, kind="ExternalInput"
    ).ap()
    outt = nc.dram_tensor(
        "outt", [NCLS, BPC], mybir.dt.float32, kind="ExternalOutput"
    ).ap()

    relu = mybir.ActivationFunctionType.Relu
    bf = mybir.dt.bfloat16
    f8 = mybir.dt.float8e3
    f32 = mybir.dt.float32

    ctx = contextlib.ExitStack()
    with ctx:
        CONST = ctx.enter_context(
            nc.sbuf_tensor("CONST", [128, CPK_BYTES], mybir.dt.uint8)
        )
        W1 = [
            CONST[:KP, 256 * k : 256 * (k + 1)].bitcast(bf) for k in range(KB)
        ]
        W2 = CONST[:, 1792:1812].bitcast(bf)
        B1 = CONST[:, 1812:1816].bitcast(f32)
        x0 = ctx.enter_context(nc.sbuf_tensor("x0", [KP, KB, NB], f8))
        # half-chunk slots: even slots hold k0..3, odd slots k4..6
        xh = [
            ctx.enter_context(nc.sbuf_tensor(f"xh{i}", [KP, KH0, NB], f8))
            for i in range(NXS)
        ]
        h1 = [
            ctx.enter_context(nc.sbuf_tensor(f"h1_{i}", [128, NB], bf))
            for i in range(NH1)
        ]
        ob = [
            ctx.enter_context(nc.sbuf_tensor(f"ob{i}", [NCLS, NB], f32))
            for i in range(NOB)
        ]
        ps1 = [
            ctx.enter_context(nc.psum_tensor(f"ps1_{i}", [128, NB], f32))
            for i in range(NPS1)
        ]
        ps2 = [
            ctx.enter_context(nc.psum_tensor(f"ps2_{i}", [NCLS, 512], f32))
            for i in range(NPS2)
        ]
        WARM = ctx.enter_context(nc.sbuf_tensor("WARM", [128, 512], f8))

        s_cpk = ctx.enter_context(nc.semaphore("s_cpk"))
        s_x0 = [
            ctx.enter_context(nc.semaphore(f"s_x0_{j}"))
            for j in range(len(C0_PIECES))
        ]
        s_xs = [ctx.enter_context(nc.semaphore(f"s_xs{i}")) for i in range(NXS)]
        s_os = [ctx.enter_context(nc.semaphore(f"s_os{i}")) for i in range(NOB)]
        s_l1 = ctx.enter_context(nc.semaphore("s_l1"))
        s_l1a = ctx.enter_context(nc.semaphore("s_l1a"))  # chunk7 s0 done
        s_l1b = ctx.enter_context(nc.semaphore("s_l1b"))  # chunk7 s1 done
        s_act = ctx.enter_context(nc.semaphore("s_act"))
        s_a7 = [ctx.enter_context(nc.semaphore(f"s_a7_{j}")) for j in range(2)]
        s_l2 = ctx.enter_context(nc.semaphore("s_l2"))
        s_cp = ctx.enter_context(nc.semaphore("s_cp"))
        s_warm = ctx.enter_context(nc.semaphore("s_warm"))

        block = ctx.enter_context(nc.Block())

        # half h of chunk n (1..7) lives in slot (2*n+h) % NXS
        xs_count = [0] * NXS
        xs_target = {}

        @block.sync
        def _(sync):
            # chunk 0 in 4 small pieces so the first matmul data lands early
            for j, (k0, k1) in enumerate(C0_PIECES):
                sync.dma_start(
                    x0[:, k0:k1, :],
                    xm[0, :, k0 * NB : k1 * NB].rearrange(
                        "p (c b) -> p c b", c=k1 - k0
                    ),
                ).then_inc(s_x0[j], 16)
            # chunk 1's halves (hh=2,3) are issued by the scalar block; they
            # still occupy slots 2,3 of the ring
            for hh in (2, 3):
                xs_count[hh % NXS] += 1
                xs_target[hh] = 16 * xs_count[hh % NXS]
            # chunks 2..7 as half-chunk DMAs into the slot ring (chunk 1's
            # halves go out on the scalar queue so their completion sems
            # don't trail the piece stream)
            for hh in range(4, 2 * NCHUNK):
                n, h = hh // 2, hh % 2
                slot = hh % NXS
                if hh >= NXS + 2:
                    # slot last read by chunk (hh - NXS) // 2's L1
                    sync.wait_ge(s_l1, (hh - NXS) // 2 + 1)
                k0, k1 = (0, KH0) if h == 0 else (KH0, KB)
                sync.dma_start(
                    xh[slot][:, : k1 - k0, :],
                    xm[n, :, k0 * NB : k1 * NB].rearrange(
                        "p (c b) -> p c b", c=k1 - k0
                    ),
                ).then_inc(s_xs[slot], 16)
                xs_count[slot] += 1
                xs_target[hh] = 16 * xs_count[slot]
            # chunks 5,6 outputs + chunk 7 half outputs (sync queue is idle
            # by then; scalar stays on relu cadence)
            for n in (NCHUNK - 3, NCHUNK - 2):
                sync.wait_ge(s_cp, 2 * (n + 1))
                sync.dma_start(
                    outt[:, n * NB : (n + 1) * NB], ob[n % NOB][:]
                ).then_inc(s_os[n % NOB], 16)
            base = (NCHUNK - 1) * NB
            sync.wait_ge(s_cp, 15)
            sync.dma_start(
                outt[:, base : base + 512], ob[(NCHUNK - 1) % NOB][:, :512]
            ).then_inc(s_os[(NCHUNK - 1) % NOB], 16)
            sync.wait_ge(s_cp, 16)
            sync.dma_start(
                outt[:, base + 512 : base + 1024],
                ob[(NCHUNK - 1) % NOB][:, 512:],
            ).then_inc(s_os[(NCHUNK - 1) % NOB], 16)

        def xsrc(n, k):
            if n == 0:
                return x0[:, k, :]
            h = 0 if k < KH0 else 1
            slot = (2 * n + h) % NXS
            return xh[slot][:, k - (0 if h == 0 else KH0), :]

        def xwait(tensor, n, h):
            hh = 2 * n + h
            slot = hh % NXS
            tensor.wait_ge(s_xs[slot], xs_target[hh])

        def l2_mm(hsl, psl, psi, n):
            # second-layer matmul: out.T slice <- W2T.T @ h1 slice
            nc.tensor.matmul(
                ps2[psi][:, psl],
                W2[:],
                h1[n % NH1][:, hsl],
                start=True,
                stop=True,
            ).then_inc(s_l2, 1)

        @block.tensor
        def _(tensor):
            # PE warm-up at mid p-state while the first x piece streams in.
            # Stationary is a bf16 view of the (memset) WARM tile.
            tensor.wait_ge(s_warm, 1)
            warm_st = WARM.bitcast(bf)
            for i in range(NWARM):
                nc.tensor.matmul(
                    ps1[0][:, :512],
                    warm_st[:, :128],
                    WARM[:, :],
                    start=(i == 0),
                    stop=(i == NWARM - 1),
                )
            # ---- chunks 0..6: k-major over both 512-col subtiles ----
            for n in range(NCHUNK - 1):
                p1 = ps1[n % NPS1]
                if n == 0:
                    tensor.wait_ge(s_cpk, 16)
                last = None
                for k in range(KB):
                    if n == 0:
                        for j, (k0, _k1) in enumerate(C0_PIECES):
                            if k == k0:
                                tensor.wait_ge(s_x0[j], 16)
                        if k == 2:
                            tensor.wait_ge(s_cpk, 32)  # rest of the consts
                    elif k == KH0:
                        xwait(tensor, n, 1)
                    if k == KB - 1:
                        # hoisted deps for the L2(n-1) pair, ps2 ring and the
                        # next chunk's first x half, so the boundary matmuls
                        # and the L2 pair are wait-free (LDW prefetch works)
                        if n >= 1:
                            tensor.wait_ge(s_act, n)
                        if n >= 2:
                            tensor.wait_ge(s_cp, 2 * (n - 1))
                        xwait(tensor, n + 1, 0)
                    for s in range(2):
                        ssl = slice(s * 512, (s + 1) * 512)
                        last = nc.tensor.matmul(
                            p1[:, ssl],
                            W1[k],
                            xsrc(n, k)[:, ssl],
                            start=(k == 0),
                            stop=(k == KB - 1),
                        )
                last.then_inc(s_l1, 1)
                if n >= 1:
                    for s in range(2):
                        idx = 2 * (n - 1) + s
                        l2_mm(
                            slice(s * 512, (s + 1) * 512),
                            slice(0, 512),
                            idx % NPS2,
                            n - 1,
                        )

            # ---- chunk 7: s-major halves (PSUM-bank aligned) ----
            n = NCHUNK - 1
            p1 = ps1[n % NPS1]
            tensor.wait_ge(s_act, n - 2)  # ps1 ring (relu(4) done)
            xwait(tensor, n, 1)  # second half data (hoisted)
            for k in range(KB):
                last = nc.tensor.matmul(
                    p1[:, 0:512],
                    W1[k],
                    xsrc(n, k)[:, 0:512],
                    start=(k == 0),
                    stop=(k == KB - 1),
                )
            last.then_inc(s_l1a, 1)
            # L2(6) pair slots in here (relu(6) finished during s0)
            tensor.wait_ge(s_act, n)
            tensor.wait_ge(s_cp, 2 * (n - 1))
            for s in range(2):
                idx = 2 * (n - 1) + s
                l2_mm(
                    slice(s * 512, (s + 1) * 512),
                    slice(0, 512),
                    idx % NPS2,
                    n - 1,
                )
            for k in range(KB):
                if k == KB - 1:
                    # hoisted deps for L2(7a): relu7a + ps2[0] ring
                    tensor.wait_ge(s_a7[0], 1)
                    tensor.wait_ge(s_cp, 13)
                last = nc.tensor.matmul(
                    p1[:, 512:1024],
                    W1[k],
                    xsrc(n, k)[:, 512:1024],
                    start=(k == 0),
                    stop=(k == KB - 1),
                )
            last.then_inc(s_l1b, 1)
            # L2(7a) on cols 0..511 (relu7a ran during s1)
            l2_mm(slice(0, 512), slice(0, 512), 0, n)
            tensor.wait_ge(s_a7[1], 1)
            tensor.wait_ge(s_cp, 14)  # ps2[1] freed (copy of idx 13)
            # L2(7b) on cols 512..1023
            l2_mm(slice(512, 1024), slice(0, 512), 1, n)

        @block.scalar
        def _(scalar):
            # consts split: W1[k0..k1] first so L1(0) k0 gates on a small DMA
            scalar.dma_start(
                CONST[:, :CPK_SPLIT], cpk[:, :CPK_SPLIT]
            ).then_inc(s_cpk, 16)
            scalar.dma_start(
                CONST[:, CPK_SPLIT:], cpk[:, CPK_SPLIT:]
            ).then_inc(s_cpk, 16)
            # chunk 1's x halves ride the scalar queue (see sync block)
            for hh in (2, 3):
                k0, k1 = (0, KH0) if hh % 2 == 0 else (KH0, KB)
                scalar.dma_start(
                    xh[hh % NXS][:, : k1 - k0, :],
                    xm[1, :, k0 * NB : k1 * NB].rearrange(
                        "p (c b) -> p c b", c=k1 - k0
                    ),
                ).then_inc(s_xs[hh % NXS], 16)
            for n in range(NCHUNK - 1):
                if n >= NH1:
                    scalar.wait_ge(s_l2, 2 * (n - NH1) + 2)
                scalar.wait_ge(s_l1, n + 1)
                nc.scalar.activation(
                    h1[n % NH1][:], ps1[n % NPS1][:], relu, bias=B1[:]
                ).then_inc(s_act, 1)
                if n >= 2:
                    # lagged output DMA for chunk n-2 (chunks 0..4)
                    scalar.wait_ge(s_cp, 2 * (n - 1))
                    scalar.dma_start(
                        outt[:, (n - 2) * NB : (n - 1) * NB],
                        ob[(n - 2) % NOB][:],
                    ).then_inc(s_os[(n - 2) % NOB], 16)
            # chunk 7 half relus (bank A then bank B of ps1[1])
            n = NCHUNK - 1
            p1 = ps1[n % NPS1]
            scalar.wait_ge(s_l2, 10)  # h1[1] free (L2(4) done)
            scalar.wait_ge(s_l1a, 1)
            nc.scalar.activation(
                h1[n % NH1][:, 0:512], p1[:, 0:512], relu, bias=B1[:]
            ).then_inc(s_a7[0], 1)
            scalar.wait_ge(s_l1b, 1)
            nc.scalar.activation(
                h1[n % NH1][:, 512:1024], p1[:, 512:1024], relu, bias=B1[:]
            ).then_inc(s_a7[1], 1)

        @block.vector
        def _(vector):
            # initialize the warm-up operand first (nonzero so the PE power
            # ramp is actually exercised); vector is idle at block start
            nc.vector.memset(WARM[:], 2.5).then_inc(s_warm, 1)
            # chunks 0..6: two 512-col copies each
            for n in range(NCHUNK - 1):
                for s in range(2):
                    idx = 2 * n + s
                    vector.wait_ge(s_l2, idx + 1)
                    if s == 0 and n >= NOB:
                        vector.wait_ge(s_os[n % NOB], 16 * (n // NOB))
                    ssl = slice(s * 512, (s + 1) * 512)
                    nc.vector.tensor_copy(
                        ob[n % NOB][:, ssl], ps2[idx % NPS2][:, 0:512]
                    ).then_inc(s_cp, 1)
            # chunk 7 halves
            n = NCHUNK - 1
            vector.wait_ge(s_os[n % NOB], 32)  # ob[1] freed (chunks 1,4 out)
            vector.wait_ge(s_l2, 15)
            nc.vector.tensor_copy(
                ob[n % NOB][:, 0:512], ps2[0][:, 0:512]
            ).then_inc(s_cp, 1)
            vector.wait_ge(s_l2, 16)
            nc.vector.tensor_copy(
                ob[n % NOB][:, 512:1024], ps2[1][:, 0:512]
            ).then_inc(s_cp, 1)

    nc.compile()
    return nc


def _get_module():
    nc = _CACHE.get("nc")
    if nc is None:
        nc = _build_module()
        _CACHE["nc"] = nc
    return nc


def _prepare_inputs(x, conv_w, w1, b1, w2, b2):
    x = np.asarray(x, dtype=np.float32)
    conv_w = np.asarray(conv_w, dtype=np.float32)
    w1 = np.asarray(w1, dtype=np.float32)
    b1 = np.asarray(b1, dtype=np.float32)
    w2 = np.asarray(w2, dtype=np.float32)
    b2 = np.asarray(b2, dtype=np.float32)

    # Fold the 3x3 cross-correlation into w1.
    w1im = w1.reshape(HID, OUT_HW, OUT_HW)
    w1_eff = np.zeros((HID, IMG, IMG), np.float32)
    for di in range(KSZ):
        for dj in range(KSZ):
            w1_eff[:, di : di + OUT_HW, dj : dj + OUT_HW] += conv_w[di, dj] * w1im

    # 1/XSCALE folded into W1 (exact: power-of-2 scale on bf16)
    w1t_pad = np.zeros((FEAT, HPAD), _BF16)
    w1t_pad[:, :HID] = (w1_eff.reshape(HID, FEAT).T / XSCALE).astype(_BF16)
    b1_pad = np.zeros(HPAD, np.float32)
    b1_pad[:HID] = b1
    b1_pad[HID] = 1.0  # h1 row 100 == relu(0+1) == 1: carries b2
    w2t_pad = np.zeros((HPAD, NCLS), _BF16)
    w2t_pad[:HID, :] = w2.T.astype(_BF16)
    w2t_pad[HID, :] = b2.astype(_BF16)

    # blocked W1: partitions 0..111 hold w1t_pad[k*112 + p, :] at col k
    w1m_host = np.zeros((128, KB * HPAD), _BF16)
    w1m_host[:KP] = np.ascontiguousarray(
        w1t_pad.reshape(KB, KP, HPAD).transpose(1, 0, 2)
    ).reshape(KP, KB * HPAD)

    cpk = np.empty((128, CPK_BYTES), np.uint8)
    cpk[:, :1792] = w1m_host.view(np.uint8)
    cpk[:, 1792:1812] = w2t_pad.view(np.uint8)
    cpk[:, 1812:1816] = b1_pad.reshape(128, 1).view(np.uint8)

    xb = (x * XSCALE).astype(_F8)
    # xm[n, p, k*NB+b] = xq[n*NB+b, k*112+p]
    xcores = xb.reshape(N_CORES, NCHUNK, NB, KB, KP)
    xm_all = np.ascontiguousarray(xcores.transpose(0, 1, 4, 3, 2)).reshape(
        N_CORES, NCHUNK, KP, XBYTES
    )

    return [{"xm": xm_all[i], "cpk": cpk} for i in range(N_CORES)]


def _ensure_accel_backend():
    # If the caller pinned JAX_PLATFORMS=cpu, the axon/neuron PJRT devices
    # are invisible and the SPMD run would fail; undo that for this process.
    import os

    import jax

    try:
        if all(d.platform == "cpu" for d in jax.devices()):
            if os.environ.get("JAX_PLATFORMS"):
                os.environ["JAX_PLATFORMS"] = ""
                from jax.extend import backend as _jeb

                _jeb.clear_backends()
    except Exception:
        pass


def _run_device(in_maps, trace=False, trace_cores=None):
    _ensure_accel_backend()
    from concourse.bass_utils import run_bass_kernel_spmd

    nc = _get_module()
    return run_bass_kernel_spmd(
        nc,
        in_maps,
        core_ids=list(range(N_CORES)),
        trace=trace,
        trace_cores=trace_cores,
    )


def kernel(x, conv_w, w1, b1, w2, b2):
    in_maps = _prepare_inputs(x, conv_w, w1, b1, w2, b2)
    res = _run_device(in_maps)
    out = np.empty((B, NCLS), np.float32)
    for i in range(N_CORES):
        out[i * BPC : (i + 1) * BPC] = res.results[i]["outt"].T
    return out


# revision 22
# speedup vs baseline: 1.1744x; 1.1744x over previous
"""Trainium2 Bass kernel for DigitConvolutionalModel (self-contained).

Model: out = relu(conv3x3(x) @ w1.T + b1) @ w2.T + b2, x: [65536, 784] f32.

The 3x3 valid cross-correlation is linear in x, so it is folded into the
first linear layer on the host, giving a 2-layer MLP:
out = relu(x @ W1_eff.T + b1) @ w2.T + b2.

Sharding: pure data parallelism - batch split 8 ways (8192 rows/core),
weights replicated; no collectives. Per core the kernel computes
out.T [10, 8192] with batch on the matmul free dim and features on
partitions. Host casts 2*x to fp8 E3M4 (scale 2 halves the subnormal
fraction; 1/2 folded into bf16 W1) and lays it out in blocked SBUF tile
order. Features are tiled 7 x 112 (784 exactly), so there is no K=16
remainder pass: per 1024-col chunk the PE runs 14 L1 matmuls + 2 L2
matmuls, all N=512 at ~216 ns (fp8e3 moving = 1 col/cycle @2.4GHz).

Schedule notes (from trace analysis):
 - PE p-state reaches 2.4 GHz only after ~8-9 us of busy time; warm-up
   matmuls run from block start and bridge into the first data with no
   idle gap (a gap resets progress and costs ~2x matmuls for a while).
 - DMA completion semaphores trail the data descriptors by 1.5-5 us when
   the queue is deep, so the stream uses half-chunk granularity and the
   PE never waits on a DMA issued less than ~2 chunks earlier; chunk 0
   streams as 4 small pieces (1+1+2+3 k-blocks) consumed k-by-k.
 - Tensor-queue semaphore waits are hoisted a couple of matmuls before
   the group boundary they guard so LDWEIGHTS prefetch is not blocked
   (kills ~100 ns per stationary switch).
 - Last chunk is processed s0[0:512] then s1[512:1024] (PSUM-bank
   aligned: the relu of one half may not touch the bank the PE is still
   writing), with relu/L2/copy/output pipelined into the PE tail; final
   output goes out as two half DMAs on the (idle by then) Sync queue.
 - hidden dim padded 100 -> 128 with zero weight columns; b1_pad[100]=1
   makes h1 row 100 == 1.0 and W2T row 100 = b2, folding the second-layer
   bias into the second matmul.
"""

import sys

import numpy as np

if "/opt/trn_rl_repo" not in sys.path:
    sys.path.insert(0, "/opt/trn_rl_repo")

import ml_dtypes

B = 65536
IMG = 28
KSZ = 3
OUT_HW = IMG - KSZ + 1  # 26
FLAT = OUT_HW * OUT_HW  # 676
HID = 100
NCLS = 10
FEAT = IMG * IMG  # 784

N_CORES = 8
BPC = B // N_CORES  # 8192 batch rows per core
KB = 7  # feature k-blocks
KP = FEAT // KB  # 112 features per block
KH0 = 4  # k-blocks in each chunk's first half DMA
HPAD = 128
NB = 1024  # batch rows per chunk
NCHUNK = BPC // NB  # 8
XBYTES = KB * NB  # 7168 bytes per partition per chunk

NXS = 8  # half-chunk x slot ring (chunks 1-7 -> 14 halves)
NPS1 = 3
NPS2 = 2
NH1 = 3
NOB = 3
NWARM = 7
CPK_BYTES = 1816
CPK_SPLIT = 512  # W1[k0..k1] in the first consts DMA, rest in the second
C0_PIECES = ((0, 1), (1, 2), (2, 4), (4, 7))  # chunk-0 k-block pieces

_BF16 = ml_dtypes.bfloat16
_F8 = ml_dtypes.float8_e3m4
XSCALE = 2.0  # x pre-scale before fp8 (1/XSCALE folded into W1)
_CACHE = {}


def _enable_ldw_opt():
    # Rebind concourse.bass_utils.bir_verify_and_optimise with walrus's
    # --enable-ldw-opt flipped on: consecutive matmuls sharing a stationary
    # tensor reuse the loaded weights. Falls back silently if the source no
    # longer matches.
    if _CACHE.get("ldw_patched"):
        return
    _CACHE["ldw_patched"] = True
    try:
        import inspect

        import concourse.bass_utils as bu

        src = inspect.getsource(bu.bir_verify_and_optimise)
        if "--enable-ldw-opt=false" in src:
            src = src.replace("--enable-ldw-opt=false", "--enable-ldw-opt=true")
            exec(compile(src, bu.__file__, "exec"), bu.__dict__)
    except Exception:
        pass


def _build_module():
    import contextlib

    from concourse import bacc, mybir

    _enable_ldw_opt()

    nc = bacc.Bacc(
        "TRN2", target_bir_lowering=False, debug=False, num_devices=N_CORES
    )
    xm = nc.dram_tensor(
        "xm", [NCHUNK, KP, XBYTES], mybir.dt.float8e3, kind="ExternalInput"
    ).ap()
    cpk = nc.dram_tensor(
        "cpk", [128, CPK_BYTES], mybir.dt.uint8, kind="ExternalInput"
    ).ap()
    outt = nc.dram_tensor(
        "outt", [NCLS, BPC], mybir.dt.float32, kind="ExternalOutput"
    ).ap()

    relu = mybir.ActivationFunctionType.Relu
    bf = mybir.dt.bfloat16
    f8 = mybir.dt.float8e3
    f32 = mybir.dt.float32

    ctx = contextlib.ExitStack()
    with ctx:
        CONST = ctx.enter_context(
            nc.sbuf_tensor("CONST", [128, CPK_BYTES], mybir.dt.uint8)
        )
        W1 = [
            CONST[:KP, 256 * k : 256 * (k + 1)].bitcast(bf) for k in range(KB)
        ]
        W2 = CONST[:, 1792:1812].bitcast(bf)
        B1 = CONST[:, 1812:1816].bitcast(f32)
        x0 = ctx.enter_context(nc.sbuf_tensor("x0", [KP, KB, NB], f8))
        # half-chunk slots: even slots hold k0..3, odd slots k4..6
        xh = [
            ctx.enter_context(nc.sbuf_tensor(f"xh{i}", [KP, KH0, NB], f8))
            for i in range(NXS)
        ]
        h1 = [
            ctx.enter_context(nc.sbuf_tensor(f"h1_{i}", [128, NB], bf))
            for i in range(NH1)
        ]
        ob = [
            ctx.enter_context(nc.sbuf_tensor(f"ob{i}", [NCLS, NB], f32))
            for i in range(NOB)
        ]
        ps1 = [
            ctx.enter_context(nc.psum_tensor(f"ps1_{i}", [128, NB], f32))
            for i in range(NPS1)
        ]
        ps2 = [
            ctx.enter_context(nc.psum_tensor(f"ps2_{i}", [NCLS, 512], f32))
            for i in range(NPS2)
        ]
        WARM = ctx.enter_context(nc.sbuf_tensor("WARM", [128, 512], f8))

        s_cpk = ctx.enter_context(nc.semaphore("s_cpk"))
        s_cpk2 = ctx.enter_context(nc.semaphore("s_cpk2"))
        s_x0 = [
            ctx.enter_context(nc.semaphore(f"s_x0_{j}"))
            for j in range(len(C0_PIECES))
        ]
        s_xs = [ctx.enter_context(nc.semaphore(f"s_xs{i}")) for i in range(NXS)]
        s_os = [ctx.enter_context(nc.semaphore(f"s_os{i}")) for i in range(NOB)]
        s_l1 = ctx.enter_context(nc.semaphore("s_l1"))
        s_l1a = ctx.enter_context(nc.semaphore("s_l1a"))  # chunk7 s0 done
        s_l1b = ctx.enter_context(nc.semaphore("s_l1b"))  # chunk7 s1 done
        s_act = ctx.enter_context(nc.semaphore("s_act"))
        s_a7 = [ctx.enter_context(nc.semaphore(f"s_a7_{j}")) for j in range(2)]
        s_l2 = ctx.enter_context(nc.semaphore("s_l2"))
        s_cp = ctx.enter_context(nc.semaphore("s_cp"))
        s_warm = ctx.enter_context(nc.semaphore("s_warm"))

        block = ctx.enter_context(nc.Block())

        # half h of chunk n (1..7) lives in slot (2*n+h) % NXS
        xs_count = [0] * NXS
        xs_target = {}

        @block.sync
        def _(sync):
            # chunk 0 in 4 small pieces so the first matmul data lands early
            for j, (k0, k1) in enumerate(C0_PIECES):
                sync.dma_start(
                    x0[:, k0:k1, :],
                    xm[0, :, k0 * NB : k1 * NB].rearrange(
                        "p (c b) -> p c b", c=k1 - k0
                    ),
                ).then_inc(s_x0[j], 16)
            # chunk 1's halves (hh=2,3) are issued by the scalar block; they
            # still occupy slots 2,3 of the ring
            for hh in (2, 3):
                xs_count[hh % NXS] += 1
                xs_target[hh] = 16 * xs_count[hh % NXS]
            # chunks 2..7 as half-chunk DMAs into the slot ring (chunk 1's
            # halves go out on the scalar queue so their completion sems
            # don't trail the piece stream)
            for hh in range(4, 2 * NCHUNK):
                n, h = hh // 2, hh % 2
                slot = hh % NXS
                if hh >= NXS + 2:
                    # slot last read by chunk (hh - NXS) // 2's L1
                    sync.wait_ge(s_l1, (hh - NXS) // 2 + 1)
                k0, k1 = (0, KH0) if h == 0 else (KH0, KB)
                sync.dma_start(
                    xh[slot][:, : k1 - k0, :],
                    xm[n, :, k0 * NB : k1 * NB].rearrange(
                        "p (c b) -> p c b", c=k1 - k0
                    ),
                ).then_inc(s_xs[slot], 16)
                xs_count[slot] += 1
                xs_target[hh] = 16 * xs_count[slot]
            # chunks 5,6 outputs + chunk 7 half outputs (sync queue is idle
            # by then; scalar stays on relu cadence)
            for n in (NCHUNK - 3, NCHUNK - 2):
                sync.wait_ge(s_cp, 2 * (n + 1))
                sync.dma_start(
                    outt[:, n * NB : (n + 1) * NB], ob[n % NOB][:]
                ).then_inc(s_os[n % NOB], 16)
            base = (NCHUNK - 1) * NB
            sync.wait_ge(s_cp, 15)
            sync.dma_start(
                outt[:, base : base + 512], ob[(NCHUNK - 1) % NOB][:, :512]
            ).then_inc(s_os[(NCHUNK - 1) % NOB], 16)
            sync.wait_ge(s_cp, 16)
            sync.dma_start(
                outt[:, base + 512 : base + 1024],
                ob[(NCHUNK - 1) % NOB][:, 512:],
            ).then_inc(s_os[(NCHUNK - 1) % NOB], 16)

        def xsrc(n, k):
            if n == 0:
                return x0[:, k, :]
            h = 0 if k < KH0 else 1
            slot = (2 * n + h) % NXS
            return xh[slot][:, k - (0 if h == 0 else KH0), :]

        def xwait(tensor, n, h):
            hh = 2 * n + h
            slot = hh % NXS
            tensor.wait_ge(s_xs[slot], xs_target[hh])

        def l2_mm(hsl, psl, psi, n):
            # second-layer matmul: out.T slice <- W2T.T @ h1 slice
            nc.tensor.matmul(
                ps2[psi][:, psl],
                W2[:],
                h1[n % NH1][:, hsl],
                start=True,
                stop=True,
            ).then_inc(s_l2, 1)

        @block.tensor
        def _(tensor):
            # PE warm-up at mid p-state while the first x piece streams in.
            # Stationary is a bf16 view of the (memset) WARM tile.
            tensor.wait_ge(s_warm, 1)
            warm_st = WARM.bitcast(bf)
            for i in range(NWARM):
                nc.tensor.matmul(
                    ps1[0][:, :512],
                    warm_st[:, :128],
                    WARM[:, :],
                    start=(i == 0),
                    stop=(i == NWARM - 1),
                )
            # ---- chunks 0..6: k-major over both 512-col subtiles ----
            for n in range(NCHUNK - 1):
                p1 = ps1[n % NPS1]
                if n == 0:
                    tensor.wait_ge(s_cpk, 16)
                last = None
                for k in range(KB):
                    if n == 0:
                        for j, (k0, _k1) in enumerate(C0_PIECES):
                            if k == k0:
                                tensor.wait_ge(s_x0[j], 16)
                        if k == 2:
                            tensor.wait_ge(s_cpk2, 16)  # rest of the consts
                    elif k == KH0:
                        xwait(tensor, n, 1)
                    if k == KB - 1:
                        # hoisted deps for the L2(n-1) pair, ps2 ring and the
                        # next chunk's first x half, so the boundary matmuls
                        # and the L2 pair are wait-free (LDW prefetch works)
                        if n >= 1:
                            tensor.wait_ge(s_act, n)
                        if n >= 2:
                            tensor.wait_ge(s_cp, 2 * (n - 1))
                        xwait(tensor, n + 1, 0)
                    for s in range(2):
                        ssl = slice(s * 512, (s + 1) * 512)
                        last = nc.tensor.matmul(
                            p1[:, ssl],
                            W1[k],
                            xsrc(n, k)[:, ssl],
                            start=(k == 0),
                            stop=(k == KB - 1),
                        )
                last.then_inc(s_l1, 1)
                if n >= 1:
                    for s in range(2):
                        idx = 2 * (n - 1) + s
                        l2_mm(
                            slice(s * 512, (s + 1) * 512),
                            slice(0, 512),
                            idx % NPS2,
                            n - 1,
                        )

            # ---- chunk 7: s-major halves (PSUM-bank aligned) ----
            n = NCHUNK - 1
            p1 = ps1[n % NPS1]
            tensor.wait_ge(s_act, n - 2)  # ps1 ring (relu(4) done)
            xwait(tensor, n, 1)  # second half data (hoisted)
            for k in range(KB):
                last = nc.tensor.matmul(
                    p1[:, 0:512],
                    W1[k],
                    xsrc(n, k)[:, 0:512],
                    start=(k == 0),
                    stop=(k == KB - 1),
                )
            last.then_inc(s_l1a, 1)
            # L2(6) pair slots in here (relu(6) finished during s0)
            tensor.wait_ge(s_act, n)
            tensor.wait_ge(s_cp, 2 * (n - 1))
            for s in range(2):
                idx = 2 * (n - 1) + s
                l2_mm(
                    slice(s * 512, (s + 1) * 512),
                    slice(0, 512),
                    idx % NPS2,
                    n - 1,
                )
            for k in range(KB):
                if k == KB - 1:
                    # hoisted deps for L2(7a): relu7a + ps2[0] ring
                    tensor.wait_ge(s_a7[0], 1)
                    tensor.wait_ge(s_cp, 13)
                last = nc.tensor.matmul(
                    p1[:, 512:1024],
                    W1[k],
                    xsrc(n, k)[:, 512:1024],
                    start=(k == 0),
                    stop=(k == KB - 1),
                )
            last.then_inc(s_l1b, 1)
            # L2(7a) on cols 0..511 (relu7a ran during s1)
            l2_mm(slice(0, 512), slice(0, 512), 0, n)
            tensor.wait_ge(s_a7[1], 1)
            tensor.wait_ge(s_cp, 14)  # ps2[1] freed (copy of idx 13)
            # L2(7b) on cols 512..1023
            l2_mm(slice(512, 1024), slice(0, 512), 1, n)

        @block.scalar
        def _(scalar):
            # consts split: W1[k0..k1] first so L1(0) k0 gates on a small DMA
            scalar.dma_start(
                CONST[:, :CPK_SPLIT], cpk[:, :CPK_SPLIT]
            ).then_inc(s_cpk, 16)
            scalar.dma_start(
                CONST[:, CPK_SPLIT:], cpk[:, CPK_SPLIT:]
            ).then_inc(s_cpk2, 16)
            # chunk 1's x halves ride the scalar queue (see sync block)
            for hh in (2, 3):
                k0, k1 = (0, KH0) if hh % 2 == 0 else (KH0, KB)
                scalar.dma_start(
                    xh[hh % NXS][:, : k1 - k0, :],
                    xm[1, :, k0 * NB : k1 * NB].rearrange(
                        "p (c b) -> p c b", c=k1 - k0
                    ),
                ).then_inc(s_xs[hh % NXS], 16)
            for n in range(NCHUNK - 1):
                if n >= NH1:
                    scalar.wait_ge(s_l2, 2 * (n - NH1) + 2)
                scalar.wait_ge(s_l1, n + 1)
                nc.scalar.activation(
                    h1[n % NH1][:], ps1[n % NPS1][:], relu, bias=B1[:]
                ).then_inc(s_act, 1)
                if n >= 2:
                    # lagged output DMA for chunk n-2 (chunks 0..4)
                    scalar.wait_ge(s_cp, 2 * (n - 1))
                    scalar.dma_start(
                        outt[:, (n - 2) * NB : (n - 1) * NB],
                        ob[(n - 2) % NOB][:],
                    ).then_inc(s_os[(n - 2) % NOB], 16)
            # chunk 7 half relus (bank A then bank B of ps1[1])
            n = NCHUNK - 1
            p1 = ps1[n % NPS1]
            scalar.wait_ge(s_l2, 10)  # h1[1] free (L2(4) done)
            scalar.wait_ge(s_l1a, 1)
            nc.scalar.activation(
                h1[n % NH1][:, 0:512], p1[:, 0:512], relu, bias=B1[:]
            ).then_inc(s_a7[0], 1)
            scalar.wait_ge(s_l1b, 1)
            nc.scalar.activation(
                h1[n % NH1][:, 512:1024], p1[:, 512:1024], relu, bias=B1[:]
            ).then_inc(s_a7[1], 1)

        @block.vector
        def _(vector):
            # initialize the warm-up operand first (nonzero so the PE power
            # ramp is actually exercised); vector is idle at block start
            nc.vector.memset(WARM[:], 2.5).then_inc(s_warm, 1)
            # chunks 0..6: two 512-col copies each
            for n in range(NCHUNK - 1):
                for s in range(2):
                    idx = 2 * n + s
                    vector.wait_ge(s_l2, idx + 1)
                    if s == 0 and n >= NOB:
                        vector.wait_ge(s_os[n % NOB], 16 * (n // NOB))
                    ssl = slice(s * 512, (s + 1) * 512)
                    nc.vector.tensor_copy(
                        ob[n % NOB][:, ssl], ps2[idx % NPS2][:, 0:512]
                    ).then_inc(s_cp, 1)
            # chunk 7 halves
            n = NCHUNK - 1
            vector.wait_ge(s_os[n % NOB], 32)  # ob[1] freed (chunks 1,4 out)
            vector.wait_ge(s_l2, 15)
            nc.vector.tensor_copy(
                ob[n % NOB][:, 0:512], ps2[0][:, 0:512]
            ).then_inc(s_cp, 1)
            vector.wait_ge(s_l2, 16)
            nc.vector.tensor_copy(
                ob[n % NOB][:, 512:1024], ps2[1][:, 0:512]
            ).then_inc(s_cp, 1)

    nc.compile()
    return nc


def _get_module():
    nc = _CACHE.get("nc")
    if nc is None:
        nc = _build_module()
        _CACHE["nc"] = nc
    return nc


def _prepare_inputs(x, conv_w, w1, b1, w2, b2):
    x = np.asarray(x, dtype=np.float32)
    conv_w = np.asarray(conv_w, dtype=np.float32)
    w1 = np.asarray(w1, dtype=np.float32)
    b1 = np.asarray(b1, dtype=np.float32)
    w2 = np.asarray(w2, dtype=np.float32)
    b2 = np.asarray(b2, dtype=np.float32)

    # Fold the 3x3 cross-correlation into w1.
    w1im = w1.reshape(HID, OUT_HW, OUT_HW)
    w1_eff = np.zeros((HID, IMG, IMG), np.float32)
    for di in range(KSZ):
        for dj in range(KSZ):
            w1_eff[:, di : di + OUT_HW, dj : dj + OUT_HW] += conv_w[di, dj] * w1im

    # 1/XSCALE folded into W1 (exact: power-of-2 scale on bf16)
    w1t_pad = np.zeros((FEAT, HPAD), _BF16)
    w1t_pad[:, :HID] = (w1_eff.reshape(HID, FEAT).T / XSCALE).astype(_BF16)
    b1_pad = np.zeros(HPAD, np.float32)
    b1_pad[:HID] = b1
    b1_pad[HID] = 1.0  # h1 row 100 == relu(0+1) == 1: carries b2
    w2t_pad = np.zeros((HPAD, NCLS), _BF16)
    w2t_pad[:HID, :] = w2.T.astype(_BF16)
    w2t_pad[HID, :] = b2.astype(_BF16)

    # blocked W1: partitions 0..111 hold w1t_pad[k*112 + p, :] at col k
    w1m_host = np.zeros((128, KB * HPAD), _BF16)
    w1m_host[:KP] = np.ascontiguousarray(
        w1t_pad.reshape(KB, KP, HPAD).transpose(1, 0, 2)
    ).reshape(KP, KB * HPAD)

    cpk = np.empty((128, CPK_BYTES), np.uint8)
    cpk[:, :1792] = w1m_host.view(np.uint8)
    cpk[:, 1792:1812] = w2t_pad.view(np.uint8)
    cpk[:, 1812:1816] = b1_pad.reshape(128, 1).view(np.uint8)

    xb = (x * XSCALE).astype(_F8)
    # xm[n, p, k*NB+b] = xq[n*NB+b, k*112+p]
    xcores = xb.reshape(N_CORES, NCHUNK, NB, KB, KP)
    xm_all = np.ascontiguousarray(xcores.transpose(0, 1, 4, 3, 2)).reshape(
        N_CORES, NCHUNK, KP, XBYTES
    )

    return [{"xm": xm_all[i], "cpk": cpk} for i in range(N_CORES)]


def _ensure_accel_backend():
    # If the caller pinned JAX_PLATFORMS=cpu, the axon/neuron PJRT devices
    # are invisible and the SPMD run would fail; undo that for this process.
    import os

    import jax

    try:
        if all(d.platform == "cpu" for d in jax.devices()):
            if os.environ.get("JAX_PLATFORMS"):
                os.environ["JAX_PLATFORMS"] = ""
                from jax.extend import backend as _jeb

                _jeb.clear_backends()
    except Exception:
        pass


def _run_device(in_maps, trace=False, trace_cores=None):
    _ensure_accel_backend()
    from concourse.bass_utils import run_bass_kernel_spmd

    nc = _get_module()
    return run_bass_kernel_spmd(
        nc,
        in_maps,
        core_ids=list(range(N_CORES)),
        trace=trace,
        trace_cores=trace_cores,
    )


def kernel(x, conv_w, w1, b1, w2, b2):
    in_maps = _prepare_inputs(x, conv_w, w1, b1, w2, b2)
    res = _run_device(in_maps)
    out = np.empty((B, NCLS), np.float32)
    for i in range(N_CORES):
        out[i * BPC : (i + 1) * BPC] = res.results[i]["outt"].T
    return out


# revision 23
# speedup vs baseline: 1.1911x; 1.0142x over previous
"""Trainium2 Bass kernel for DigitConvolutionalModel (self-contained).

Model: out = relu(conv3x3(x) @ w1.T + b1) @ w2.T + b2, x: [65536, 784] f32.

The 3x3 valid cross-correlation is linear in x, so it is folded into the
first linear layer on the host, giving a 2-layer MLP:
out = relu(x @ W1_eff.T + b1) @ w2.T + b2.

Sharding: pure data parallelism - batch split 8 ways (8192 rows/core),
weights replicated; no collectives. Per core the kernel computes
out.T [10, 8192] with batch on the matmul free dim and features on
partitions. Host casts 2*x to fp8 E3M4 (scale 2 halves the subnormal
fraction; 1/2 folded into bf16 W1) and lays it out in blocked SBUF tile
order. Features are tiled 7 x 112 (784 exactly), so there is no K=16
remainder pass: per 1024-col chunk the PE runs 14 L1 matmuls + 2 L2
matmuls, all N=512 at ~216 ns (fp8e3 moving = 1 col/cycle @2.4GHz).

Schedule notes (from trace analysis):
 - PE p-state reaches 2.4 GHz only after ~8-9 us of busy time; warm-up
   matmuls run from block start and bridge into the first data with no
   idle gap (a gap resets progress and costs ~2x matmuls for a while).
 - DMA completion semaphores trail the data descriptors by 1.5-5 us when
   the queue is deep, so the stream uses half-chunk granularity and the
   PE never waits on a DMA issued less than ~2 chunks earlier; chunk 0
   streams as 4 small pieces (1+1+2+3 k-blocks) consumed k-by-k.
 - Tensor-queue semaphore waits are hoisted a couple of matmuls before
   the group boundary they guard so LDWEIGHTS prefetch is not blocked
   (kills ~100 ns per stationary switch).
 - Last chunk is processed s0[0:512] then s1[512:1024] (PSUM-bank
   aligned: the relu of one half may not touch the bank the PE is still
   writing), with relu/L2/copy/output pipelined into the PE tail; final
   output goes out as two half DMAs on the (idle by then) Sync queue.
 - hidden dim padded 100 -> 128 with zero weight columns; b1_pad[100]=1
   makes h1 row 100 == 1.0 and W2T row 100 = b2, folding the second-layer
   bias into the second matmul.
"""

import sys

import numpy as np

if "/opt/trn_rl_repo" not in sys.path:
    sys.path.insert(0, "/opt/trn_rl_repo")

import ml_dtypes

B = 65536
IMG = 28
KSZ = 3
OUT_HW = IMG - KSZ + 1  # 26
FLAT = OUT_HW * OUT_HW  # 676
HID = 100
NCLS = 10
FEAT = IMG * IMG  # 784

N_CORES = 8
BPC = B // N_CORES  # 8192 batch rows per core
KB = 7  # feature k-blocks
KP = FEAT // KB  # 112 features per block
KH0 = 4  # k-blocks in each chunk's first half DMA
HPAD = 128
NB = 1024  # batch rows per chunk
NCHUNK = BPC // NB  # 8
XBYTES = KB * NB  # 7168 bytes per partition per chunk

NXS = 8  # half-chunk x slot ring (chunks 1-7 -> 14 halves)
NPS1 = 3
NPS2 = 2
NH1 = 3
NOB = 3
NWARM = 9
CPK_BYTES = 1816
CPK_SPLIT = 512  # W1[k0..k1] in the first consts DMA, rest in the second
C0_PIECES = ((0, 4), (4, 7))  # chunk-0 k-block pieces

_BF16 = ml_dtypes.bfloat16
_F8 = ml_dtypes.float8_e3m4
XSCALE = 2.0  # x pre-scale before fp8 (1/XSCALE folded into W1)
_CACHE = {}


def _enable_ldw_opt():
    # Rebind concourse.bass_utils.bir_verify_and_optimise with walrus's
    # --enable-ldw-opt flipped on: consecutive matmuls sharing a stationary
    # tensor reuse the loaded weights. Falls back silently if the source no
    # longer matches.
    if _CACHE.get("ldw_patched"):
        return
    _CACHE["ldw_patched"] = True
    try:
        import inspect

        import concourse.bass_utils as bu

        src = inspect.getsource(bu.bir_verify_and_optimise)
        if "--enable-ldw-opt=false" in src:
            src = src.replace("--enable-ldw-opt=false", "--enable-ldw-opt=true")
            exec(compile(src, bu.__file__, "exec"), bu.__dict__)
    except Exception:
        pass


def _build_module():
    import contextlib

    from concourse import bacc, mybir

    _enable_ldw_opt()

    nc = bacc.Bacc(
        "TRN2", target_bir_lowering=False, debug=False, num_devices=N_CORES
    )
    xm = nc.dram_tensor(
        "xm", [NCHUNK, KP, XBYTES], mybir.dt.float8e3, kind="ExternalInput"
    ).ap()
    cpk = nc.dram_tensor(
        "cpk", [128, CPK_BYTES], mybir.dt.uint8, kind="ExternalInput"
    ).ap()
    outt = nc.dram_tensor(
        "outt", [NCLS, BPC], mybir.dt.float32, kind="ExternalOutput"
    ).ap()

    relu = mybir.ActivationFunctionType.Relu
    bf = mybir.dt.bfloat16
    f8 = mybir.dt.float8e3
    f32 = mybir.dt.float32

    ctx = contextlib.ExitStack()
    with ctx:
        CONST = ctx.enter_context(
            nc.sbuf_tensor("CONST", [128, CPK_BYTES], mybir.dt.uint8)
        )
        W1 = [
            CONST[:KP, 256 * k : 256 * (k + 1)].bitcast(bf) for k in range(KB)
        ]
        W2 = CONST[:, 1792:1812].bitcast(bf)
        B1 = CONST[:, 1812:1816].bitcast(f32)
        x0 = ctx.enter_context(nc.sbuf_tensor("x0", [KP, KB, NB], f8))
        # half-chunk slots: even slots hold k0..3, odd slots k4..6
        xh = [
            ctx.enter_context(nc.sbuf_tensor(f"xh{i}", [KP, KH0, NB], f8))
            for i in range(NXS)
        ]
        h1 = [
            ctx.enter_context(nc.sbuf_tensor(f"h1_{i}", [128, NB], bf))
            for i in range(NH1)
        ]
        ob = [
            ctx.enter_context(nc.sbuf_tensor(f"ob{i}", [NCLS, NB], f32))
            for i in range(NOB)
        ]
        ps1 = [
            ctx.enter_context(nc.psum_tensor(f"ps1_{i}", [128, NB], f32))
            for i in range(NPS1)
        ]
        ps2 = [
            ctx.enter_context(nc.psum_tensor(f"ps2_{i}", [NCLS, 512], f32))
            for i in range(NPS2)
        ]
        WARM = ctx.enter_context(nc.sbuf_tensor("WARM", [128, 512], f8))

        s_cpk = ctx.enter_context(nc.semaphore("s_cpk"))
        s_cpk2 = ctx.enter_context(nc.semaphore("s_cpk2"))
        s_x0 = [
            ctx.enter_context(nc.semaphore(f"s_x0_{j}"))
            for j in range(len(C0_PIECES))
        ]
        s_xs = [ctx.enter_context(nc.semaphore(f"s_xs{i}")) for i in range(NXS)]
        s_os = [ctx.enter_context(nc.semaphore(f"s_os{i}")) for i in range(NOB)]
        s_l1 = ctx.enter_context(nc.semaphore("s_l1"))
        s_l1a = ctx.enter_context(nc.semaphore("s_l1a"))  # chunk7 s0 done
        s_l1b = ctx.enter_context(nc.semaphore("s_l1b"))  # chunk7 s1 done
        s_act = ctx.enter_context(nc.semaphore("s_act"))
        s_a7 = [ctx.enter_context(nc.semaphore(f"s_a7_{j}")) for j in range(2)]
        s_l2 = ctx.enter_context(nc.semaphore("s_l2"))
        s_cp = ctx.enter_context(nc.semaphore("s_cp"))
        s_warm = ctx.enter_context(nc.semaphore("s_warm"))

        block = ctx.enter_context(nc.Block())

        # half h of chunk n (1..7) lives in slot (2*n+h) % NXS
        xs_count = [0] * NXS
        xs_target = {}

        @block.sync
        def _(sync):
            # chunk 0 in 4 small pieces so the first matmul data lands early
            for j, (k0, k1) in enumerate(C0_PIECES):
                sync.dma_start(
                    x0[:, k0:k1, :],
                    xm[0, :, k0 * NB : k1 * NB].rearrange(
                        "p (c b) -> p c b", c=k1 - k0
                    ),
                ).then_inc(s_x0[j], 16)
            # chunks 1..7 as half-chunk DMAs into the slot ring
            for hh in range(2, 2 * NCHUNK):
                n, h = hh // 2, hh % 2
                slot = hh % NXS
                if hh >= NXS + 2:
                    # slot last read by chunk (hh - NXS) // 2's L1
                    sync.wait_ge(s_l1, (hh - NXS) // 2 + 1)
                k0, k1 = (0, KH0) if h == 0 else (KH0, KB)
                sync.dma_start(
                    xh[slot][:, : k1 - k0, :],
                    xm[n, :, k0 * NB : k1 * NB].rearrange(
                        "p (c b) -> p c b", c=k1 - k0
                    ),
                ).then_inc(s_xs[slot], 16)
                xs_count[slot] += 1
                xs_target[hh] = 16 * xs_count[slot]
            # chunks 5,6 outputs + chunk 7 half outputs (sync queue is idle
            # by then; scalar stays on relu cadence)
            for n in (NCHUNK - 3, NCHUNK - 2):
                sync.wait_ge(s_cp, 2 * (n + 1))
                sync.dma_start(
                    outt[:, n * NB : (n + 1) * NB], ob[n % NOB][:]
                ).then_inc(s_os[n % NOB], 16)
            base = (NCHUNK - 1) * NB
            sync.wait_ge(s_cp, 15)
            sync.dma_start(
                outt[:, base : base + 512], ob[(NCHUNK - 1) % NOB][:, :512]
            ).then_inc(s_os[(NCHUNK - 1) % NOB], 16)
            sync.wait_ge(s_cp, 16)
            sync.dma_start(
                outt[:, base + 512 : base + 1024],
                ob[(NCHUNK - 1) % NOB][:, 512:],
            ).then_inc(s_os[(NCHUNK - 1) % NOB], 16)

        def xsrc(n, k):
            if n == 0:
                return x0[:, k, :]
            h = 0 if k < KH0 else 1
            slot = (2 * n + h) % NXS
            return xh[slot][:, k - (0 if h == 0 else KH0), :]

        def xwait(tensor, n, h):
            hh = 2 * n + h
            slot = hh % NXS
            tensor.wait_ge(s_xs[slot], xs_target[hh])

        def l2_mm(hsl, psl, psi, n):
            # second-layer matmul: out.T slice <- W2T.T @ h1 slice
            nc.tensor.matmul(
                ps2[psi][:, psl],
                W2[:],
                h1[n % NH1][:, hsl],
                start=True,
                stop=True,
            ).then_inc(s_l2, 1)

        @block.tensor
        def _(tensor):
            # PE warm-up at mid p-state while the first x piece streams in.
            # Stationary is a bf16 view of the (memset) WARM tile.
            tensor.wait_ge(s_warm, 1)
            warm_st = WARM.bitcast(bf)
            for i in range(NWARM):
                nc.tensor.matmul(
                    ps1[0][:, :512],
                    warm_st[:, :128],
                    WARM[:, :],
                    start=(i == 0),
                    stop=(i == NWARM - 1),
                )
            # ---- chunks 0..6: k-major over both 512-col subtiles ----
            for n in range(NCHUNK - 1):
                p1 = ps1[n % NPS1]
                if n == 0:
                    tensor.wait_ge(s_cpk, 16)
                last = None
                for k in range(KB):
                    if n == 0:
                        for j, (k0, _k1) in enumerate(C0_PIECES):
                            if k == k0:
                                tensor.wait_ge(s_x0[j], 16)
                        if k == 2:
                            tensor.wait_ge(s_cpk2, 16)  # rest of the consts
                    elif k == KH0:
                        xwait(tensor, n, 1)
                    if k == KB - 1:
                        # hoisted deps for the L2(n-1) pair, ps2 ring and the
                        # next chunk's first x half, so the boundary matmuls
                        # and the L2 pair are wait-free (LDW prefetch works)
                        if n >= 1:
                            tensor.wait_ge(s_act, n)
                        if n >= 2:
                            tensor.wait_ge(s_cp, 2 * (n - 1))
                        xwait(tensor, n + 1, 0)
                    for s in range(2):
                        ssl = slice(s * 512, (s + 1) * 512)
                        last = nc.tensor.matmul(
                            p1[:, ssl],
                            W1[k],
                            xsrc(n, k)[:, ssl],
                            start=(k == 0),
                            stop=(k == KB - 1),
                        )
                last.then_inc(s_l1, 1)
                if n >= 1:
                    for s in range(2):
                        idx = 2 * (n - 1) + s
                        l2_mm(
                            slice(s * 512, (s + 1) * 512),
                            slice(0, 512),
                            idx % NPS2,
                            n - 1,
                        )

            # ---- chunk 7: s-major halves (PSUM-bank aligned) ----
            n = NCHUNK - 1
            p1 = ps1[n % NPS1]
            tensor.wait_ge(s_act, n - 2)  # ps1 ring (relu(4) done)
            xwait(tensor, n, 1)  # second half data (hoisted)
            for k in range(KB):
                last = nc.tensor.matmul(
                    p1[:, 0:512],
                    W1[k],
                    xsrc(n, k)[:, 0:512],
                    start=(k == 0),
                    stop=(k == KB - 1),
                )
            last.then_inc(s_l1a, 1)
            # L2(6) pair slots in here (relu(6) finished during s0)
            tensor.wait_ge(s_act, n)
            tensor.wait_ge(s_cp, 2 * (n - 1))
            for s in range(2):
                idx = 2 * (n - 1) + s
                l2_mm(
                    slice(s * 512, (s + 1) * 512),
                    slice(0, 512),
                    idx % NPS2,
                    n - 1,
                )
            for k in range(KB):
                if k == KB - 1:
                    # hoisted deps for L2(7a): relu7a + ps2[0] ring
                    tensor.wait_ge(s_a7[0], 1)
                    tensor.wait_ge(s_cp, 13)
                last = nc.tensor.matmul(
                    p1[:, 512:1024],
                    W1[k],
                    xsrc(n, k)[:, 512:1024],
                    start=(k == 0),
                    stop=(k == KB - 1),
                )
            last.then_inc(s_l1b, 1)
            # L2(7a) on cols 0..511 (relu7a ran during s1)
            l2_mm(slice(0, 512), slice(0, 512), 0, n)
            tensor.wait_ge(s_a7[1], 1)
            tensor.wait_ge(s_cp, 14)  # ps2[1] freed (copy of idx 13)
            # L2(7b) on cols 512..1023
            l2_mm(slice(512, 1024), slice(0, 512), 1, n)

        @block.scalar
        def _(scalar):
            # consts split: W1[k0..k1] first so L1(0) k0 gates on a small DMA
            scalar.dma_start(
                CONST[:, :CPK_SPLIT], cpk[:, :CPK_SPLIT]
            ).then_inc(s_cpk, 16)
            scalar.dma_start(
                CONST[:, CPK_SPLIT:], cpk[:, CPK_SPLIT:]
            ).then_inc(s_cpk2, 16)
            for n in range(NCHUNK - 1):
                if n >= NH1:
                    scalar.wait_ge(s_l2, 2 * (n - NH1) + 2)
                scalar.wait_ge(s_l1, n + 1)
                nc.scalar.activation(
                    h1[n % NH1][:], ps1[n % NPS1][:], relu, bias=B1[:]
                ).then_inc(s_act, 1)
                if n >= 2:
                    # lagged output DMA for chunk n-2 (chunks 0..4)
                    scalar.wait_ge(s_cp, 2 * (n - 1))
                    scalar.dma_start(
                        outt[:, (n - 2) * NB : (n - 1) * NB],
                        ob[(n - 2) % NOB][:],
                    ).then_inc(s_os[(n - 2) % NOB], 16)
            # chunk 7 half relus (bank A then bank B of ps1[1])
            n = NCHUNK - 1
            p1 = ps1[n % NPS1]
            scalar.wait_ge(s_l2, 10)  # h1[1] free (L2(4) done)
            scalar.wait_ge(s_l1a, 1)
            nc.scalar.activation(
                h1[n % NH1][:, 0:512], p1[:, 0:512], relu, bias=B1[:]
            ).then_inc(s_a7[0], 1)
            scalar.wait_ge(s_l1b, 1)
            nc.scalar.activation(
                h1[n % NH1][:, 512:1024], p1[:, 512:1024], relu, bias=B1[:]
            ).then_inc(s_a7[1], 1)

        @block.vector
        def _(vector):
            # initialize the warm-up operand first (nonzero so the PE power
            # ramp is actually exercised); vector is idle at block start
            nc.vector.memset(WARM[:], 2.5).then_inc(s_warm, 1)
            # chunks 0..6: two 512-col copies each
            for n in range(NCHUNK - 1):
                for s in range(2):
                    idx = 2 * n + s
                    vector.wait_ge(s_l2, idx + 1)
                    if s == 0 and n >= NOB:
                        vector.wait_ge(s_os[n % NOB], 16 * (n // NOB))
                    ssl = slice(s * 512, (s + 1) * 512)
                    nc.vector.tensor_copy(
                        ob[n % NOB][:, ssl], ps2[idx % NPS2][:, 0:512]
                    ).then_inc(s_cp, 1)
            # chunk 7 halves
            n = NCHUNK - 1
            vector.wait_ge(s_os[n % NOB], 32)  # ob[1] freed (chunks 1,4 out)
            vector.wait_ge(s_l2, 15)
            nc.vector.tensor_copy(
                ob[n % NOB][:, 0:512], ps2[0][:, 0:512]
            ).then_inc(s_cp, 1)
            vector.wait_ge(s_l2, 16)
            nc.vector.tensor_copy(
                ob[n % NOB][:, 512:1024], ps2[1][:, 0:512]
            ).then_inc(s_cp, 1)

    nc.compile()
    return nc


def _get_module():
    nc = _CACHE.get("nc")
    if nc is None:
        nc = _build_module()
        _CACHE["nc"] = nc
    return nc


def _prepare_inputs(x, conv_w, w1, b1, w2, b2):
    x = np.asarray(x, dtype=np.float32)
    conv_w = np.asarray(conv_w, dtype=np.float32)
    w1 = np.asarray(w1, dtype=np.float32)
    b1 = np.asarray(b1, dtype=np.float32)
    w2 = np.asarray(w2, dtype=np.float32)
    b2 = np.asarray(b2, dtype=np.float32)

    # Fold the 3x3 cross-correlation into w1.
    w1im = w1.reshape(HID, OUT_HW, OUT_HW)
    w1_eff = np.zeros((HID, IMG, IMG), np.float32)
    for di in range(KSZ):
        for dj in range(KSZ):
            w1_eff[:, di : di + OUT_HW, dj : dj + OUT_HW] += conv_w[di, dj] * w1im

    # 1/XSCALE folded into W1 (exact: power-of-2 scale on bf16)
    w1t_pad = np.zeros((FEAT, HPAD), _BF16)
    w1t_pad[:, :HID] = (w1_eff.reshape(HID, FEAT).T / XSCALE).astype(_BF16)
    b1_pad = np.zeros(HPAD, np.float32)
    b1_pad[:HID] = b1
    b1_pad[HID] = 1.0  # h1 row 100 == relu(0+1) == 1: carries b2
    w2t_pad = np.zeros((HPAD, NCLS), _BF16)
    w2t_pad[:HID, :] = w2.T.astype(_BF16)
    w2t_pad[HID, :] = b2.astype(_BF16)

    # blocked W1: partitions 0..111 hold w1t_pad[k*112 + p, :] at col k
    w1m_host = np.zeros((128, KB * HPAD), _BF16)
    w1m_host[:KP] = np.ascontiguousarray(
        w1t_pad.reshape(KB, KP, HPAD).transpose(1, 0, 2)
    ).reshape(KP, KB * HPAD)

    cpk = np.empty((128, CPK_BYTES), np.uint8)
    cpk[:, :1792] = w1m_host.view(np.uint8)
    cpk[:, 1792:1812] = w2t_pad.view(np.uint8)
    cpk[:, 1812:1816] = b1_pad.reshape(128, 1).view(np.uint8)

    xb = (x * XSCALE).astype(_F8)
    # xm[n, p, k*NB+b] = xq[n*NB+b, k*112+p]
    xcores = xb.reshape(N_CORES, NCHUNK, NB, KB, KP)
    xm_all = np.ascontiguousarray(xcores.transpose(0, 1, 4, 3, 2)).reshape(
        N_CORES, NCHUNK, KP, XBYTES
    )

    return [{"xm": xm_all[i], "cpk": cpk} for i in range(N_CORES)]


def _ensure_accel_backend():
    # If the caller pinned JAX_PLATFORMS=cpu, the axon/neuron PJRT devices
    # are invisible and the SPMD run would fail; undo that for this process.
    import os

    import jax

    try:
        if all(d.platform == "cpu" for d in jax.devices()):
            if os.environ.get("JAX_PLATFORMS"):
                os.environ["JAX_PLATFORMS"] = ""
                from jax.extend import backend as _jeb

                _jeb.clear_backends()
    except Exception:
        pass


def _run_device(in_maps, trace=False, trace_cores=None):
    _ensure_accel_backend()
    from concourse.bass_utils import run_bass_kernel_spmd

    nc = _get_module()
    return run_bass_kernel_spmd(
        nc,
        in_maps,
        core_ids=list(range(N_CORES)),
        trace=trace,
        trace_cores=trace_cores,
    )


def kernel(x, conv_w, w1, b1, w2, b2):
    in_maps = _prepare_inputs(x, conv_w, w1, b1, w2, b2)
    res = _run_device(in_maps)
    out = np.empty((B, NCLS), np.float32)
    for i in range(N_CORES):
        out[i * BPC : (i + 1) * BPC] = res.results[i]["outt"].T
    return out


# revision 24
# speedup vs baseline: 1.2222x; 1.0261x over previous
"""Trainium2 Bass kernel for DigitConvolutionalModel (self-contained).

Model: out = relu(conv3x3(x) @ w1.T + b1) @ w2.T + b2, x: [65536, 784] f32.

The 3x3 valid cross-correlation is linear in x, so it is folded into the
first linear layer on the host, giving a 2-layer MLP:
out = relu(x @ W1_eff.T + b1) @ w2.T + b2.

Sharding: pure data parallelism - batch split 8 ways (8192 rows/core),
weights replicated; no collectives. Per core the kernel computes
out.T [10, 8192] with batch on the matmul free dim and features on
partitions. Host casts 2*x to fp8 E3M4 (scale 2 halves the subnormal
fraction; 1/2 folded into bf16 W1) and lays it out in blocked SBUF tile
order. Features are tiled 7 x 112 (784 exactly), so there is no K=16
remainder pass: per 1024-col chunk the PE runs 14 L1 matmuls + 2 L2
matmuls, all N=512 at ~216 ns (fp8e3 moving = 1 col/cycle @2.4GHz).

Schedule notes (from trace analysis):
 - PE p-state reaches 2.4 GHz only after ~8-9 us of busy time; warm-up
   matmuls run from block start and bridge into the first data with no
   idle gap (a gap resets progress and costs ~2x matmuls for a while).
 - DMA completion semaphores trail the data descriptors by 1.5-5 us when
   the queue is deep, so the stream uses half-chunk granularity and the
   PE never waits on a DMA issued less than ~2 chunks earlier; chunk 0
   streams as 4 small pieces (1+1+2+3 k-blocks) consumed k-by-k.
 - Tensor-queue semaphore waits are hoisted a couple of matmuls before
   the group boundary they guard so LDWEIGHTS prefetch is not blocked
   (kills ~100 ns per stationary switch).
 - Last chunk is processed s0[0:512] then s1[512:1024] (PSUM-bank
   aligned: the relu of one half may not touch the bank the PE is still
   writing), with relu/L2/copy/output pipelined into the PE tail; final
   output goes out as two half DMAs on the (idle by then) Sync queue.
 - hidden dim padded 100 -> 128 with zero weight columns; b1_pad[100]=1
   makes h1 row 100 == 1.0 and W2T row 100 = b2, folding the second-layer
   bias into the second matmul.
"""

import sys

import numpy as np

if "/opt/trn_rl_repo" not in sys.path:
    sys.path.insert(0, "/opt/trn_rl_repo")

import ml_dtypes

B = 65536
IMG = 28
KSZ = 3
OUT_HW = IMG - KSZ + 1  # 26
FLAT = OUT_HW * OUT_HW  # 676
HID = 100
NCLS = 10
FEAT = IMG * IMG  # 784

N_CORES = 8
BPC = B // N_CORES  # 8192 batch rows per core
KB = 7  # feature k-blocks
KP = FEAT // KB  # 112 features per block
KH0 = 4  # k-blocks in each chunk's first half DMA
HPAD = 128
NB = 1024  # batch rows per chunk
NCHUNK = BPC // NB  # 8
XBYTES = KB * NB  # 7168 bytes per partition per chunk

NXS = 8  # half-chunk x slot ring (chunks 1-7 -> 14 halves)
NPS1 = 3
NPS2 = 2
NH1 = 3
NOB = 3
NWARM = 7
CPK_BYTES = 1816
CPK_SPLIT = 1792  # all of W1 in the first consts DMA; W2+b1 in the second
C0_PIECES = ((0, 4), (4, 7))  # chunk-0 k-block pieces

_BF16 = ml_dtypes.bfloat16
_F8 = ml_dtypes.float8_e3m4
XSCALE = 2.0  # x pre-scale before fp8 (1/XSCALE folded into W1)
_CACHE = {}


def _enable_ldw_opt():
    # Rebind concourse.bass_utils.bir_verify_and_optimise with walrus's
    # --enable-ldw-opt flipped on: consecutive matmuls sharing a stationary
    # tensor reuse the loaded weights. Falls back silently if the source no
    # longer matches.
    if _CACHE.get("ldw_patched"):
        return
    _CACHE["ldw_patched"] = True
    try:
        import inspect

        import concourse.bass_utils as bu

        src = inspect.getsource(bu.bir_verify_and_optimise)
        if "--enable-ldw-opt=false" in src:
            src = src.replace("--enable-ldw-opt=false", "--enable-ldw-opt=true")
            exec(compile(src, bu.__file__, "exec"), bu.__dict__)
    except Exception:
        pass


def _build_module():
    import contextlib

    from concourse import bacc, mybir

    _enable_ldw_opt()

    nc = bacc.Bacc(
        "TRN2", target_bir_lowering=False, debug=False, num_devices=N_CORES
    )
    xm = nc.dram_tensor(
        "xm", [NCHUNK, KP, XBYTES], mybir.dt.float8e3, kind="ExternalInput"
    ).ap()
    cpk = nc.dram_tensor(
        "cpk", [128, CPK_BYTES], mybir.dt.uint8, kind="ExternalInput"
    ).ap()
    outt = nc.dram_tensor(
        "outt", [NCLS, BPC], mybir.dt.float32, kind="ExternalOutput"
    ).ap()

    relu = mybir.ActivationFunctionType.Relu
    bf = mybir.dt.bfloat16
    f8 = mybir.dt.float8e3
    f32 = mybir.dt.float32

    ctx = contextlib.ExitStack()
    with ctx:
        CONST = ctx.enter_context(
            nc.sbuf_tensor("CONST", [128, CPK_BYTES], mybir.dt.uint8)
        )
        W1 = [
            CONST[:KP, 256 * k : 256 * (k + 1)].bitcast(bf) for k in range(KB)
        ]
        W2 = CONST[:, 1792:1812].bitcast(bf)
        B1 = CONST[:, 1812:1816].bitcast(f32)
        x0 = ctx.enter_context(nc.sbuf_tensor("x0", [KP, KB, NB], f8))
        # half-chunk slots: even slots hold k0..3, odd slots k4..6
        xh = [
            ctx.enter_context(nc.sbuf_tensor(f"xh{i}", [KP, KH0, NB], f8))
            for i in range(NXS)
        ]
        h1 = [
            ctx.enter_context(nc.sbuf_tensor(f"h1_{i}", [128, NB], bf))
            for i in range(NH1)
        ]
        ob = [
            ctx.enter_context(nc.sbuf_tensor(f"ob{i}", [NCLS, NB], f32))
            for i in range(NOB)
        ]
        ps1 = [
            ctx.enter_context(nc.psum_tensor(f"ps1_{i}", [128, NB], f32))
            for i in range(NPS1)
        ]
        ps2 = [
            ctx.enter_context(nc.psum_tensor(f"ps2_{i}", [NCLS, 512], f32))
            for i in range(NPS2)
        ]
        WARM = ctx.enter_context(nc.sbuf_tensor("WARM", [128, 512], f8))

        s_cpk = ctx.enter_context(nc.semaphore("s_cpk"))
        s_cpk2 = ctx.enter_context(nc.semaphore("s_cpk2"))
        s_x0 = [
            ctx.enter_context(nc.semaphore(f"s_x0_{j}"))
            for j in range(len(C0_PIECES))
        ]
        s_xs = [ctx.enter_context(nc.semaphore(f"s_xs{i}")) for i in range(NXS)]
        s_os = [ctx.enter_context(nc.semaphore(f"s_os{i}")) for i in range(NOB)]
        s_l1 = ctx.enter_context(nc.semaphore("s_l1"))
        s_l1a = ctx.enter_context(nc.semaphore("s_l1a"))  # chunk7 s0 done
        s_l1b = ctx.enter_context(nc.semaphore("s_l1b"))  # chunk7 s1 done
        s_act = ctx.enter_context(nc.semaphore("s_act"))
        s_a7 = [ctx.enter_context(nc.semaphore(f"s_a7_{j}")) for j in range(2)]
        s_l2 = ctx.enter_context(nc.semaphore("s_l2"))
        s_cp = ctx.enter_context(nc.semaphore("s_cp"))
        s_warm = ctx.enter_context(nc.semaphore("s_warm"))

        block = ctx.enter_context(nc.Block())

        # half h of chunk n (1..7) lives in slot (2*n+h) % NXS
        xs_count = [0] * NXS
        xs_target = {}

        @block.sync
        def _(sync):
            # chunk 0 in 2 pieces; the queue stays SHALLOW until piece 0
            # completes so its semaphore does not trail the stream (the
            # laggard DMA engine effect adds 2-3 us under a deep queue)
            for j, (k0, k1) in enumerate(C0_PIECES):
                sync.dma_start(
                    x0[:, k0:k1, :],
                    xm[0, :, k0 * NB : k1 * NB].rearrange(
                        "p (c b) -> p c b", c=k1 - k0
                    ),
                ).then_inc(s_x0[j], 16)
                if j == 0:
                    sync.wait_ge(s_x0[0], 16)
            # chunks 1..7 as half-chunk DMAs into the slot ring
            for hh in range(2, 2 * NCHUNK):
                n, h = hh // 2, hh % 2
                slot = hh % NXS
                if hh >= NXS + 2:
                    # slot last read by chunk (hh - NXS) // 2's L1
                    sync.wait_ge(s_l1, (hh - NXS) // 2 + 1)
                k0, k1 = (0, KH0) if h == 0 else (KH0, KB)
                sync.dma_start(
                    xh[slot][:, : k1 - k0, :],
                    xm[n, :, k0 * NB : k1 * NB].rearrange(
                        "p (c b) -> p c b", c=k1 - k0
                    ),
                ).then_inc(s_xs[slot], 16)
                xs_count[slot] += 1
                xs_target[hh] = 16 * xs_count[slot]
            # chunks 5,6 outputs + chunk 7 half outputs (sync queue is idle
            # by then; scalar stays on relu cadence)
            for n in (NCHUNK - 3, NCHUNK - 2):
                sync.wait_ge(s_cp, 2 * (n + 1))
                sync.dma_start(
                    outt[:, n * NB : (n + 1) * NB], ob[n % NOB][:]
                ).then_inc(s_os[n % NOB], 16)
            base = (NCHUNK - 1) * NB
            sync.wait_ge(s_cp, 15)
            sync.dma_start(
                outt[:, base : base + 512], ob[(NCHUNK - 1) % NOB][:, :512]
            ).then_inc(s_os[(NCHUNK - 1) % NOB], 16)
            sync.wait_ge(s_cp, 16)
            sync.dma_start(
                outt[:, base + 512 : base + 1024],
                ob[(NCHUNK - 1) % NOB][:, 512:],
            ).then_inc(s_os[(NCHUNK - 1) % NOB], 16)

        def xsrc(n, k):
            if n == 0:
                return x0[:, k, :]
            h = 0 if k < KH0 else 1
            slot = (2 * n + h) % NXS
            return xh[slot][:, k - (0 if h == 0 else KH0), :]

        def xwait(tensor, n, h):
            hh = 2 * n + h
            slot = hh % NXS
            tensor.wait_ge(s_xs[slot], xs_target[hh])

        def l2_mm(hsl, psl, psi, n):
            # second-layer matmul: out.T slice <- W2T.T @ h1 slice
            nc.tensor.matmul(
                ps2[psi][:, psl],
                W2[:],
                h1[n % NH1][:, hsl],
                start=True,
                stop=True,
            ).then_inc(s_l2, 1)

        @block.tensor
        def _(tensor):
            # PE warm-up at mid p-state while the first x piece streams in.
            # Stationary is a bf16 view of the (memset) WARM tile.
            tensor.wait_ge(s_warm, 1)
            warm_st = WARM.bitcast(bf)
            for i in range(NWARM):
                nc.tensor.matmul(
                    ps1[0][:, :512],
                    warm_st[:, :128],
                    WARM[:, :],
                    start=(i == 0),
                    stop=(i == NWARM - 1),
                )
            # ---- chunks 0..6: k-major over both 512-col subtiles ----
            for n in range(NCHUNK - 1):
                p1 = ps1[n % NPS1]
                if n == 0:
                    tensor.wait_ge(s_cpk, 16)
                last = None
                for k in range(KB):
                    if n == 0:
                        for j, (k0, _k1) in enumerate(C0_PIECES):
                            if k == k0:
                                tensor.wait_ge(s_x0[j], 16)
                    elif k == KH0:
                        xwait(tensor, n, 1)
                    if k == KB - 1:
                        # hoisted deps for the L2(n-1) pair, ps2 ring and the
                        # next chunk's first x half, so the boundary matmuls
                        # and the L2 pair are wait-free (LDW prefetch works)
                        if n == 0:
                            tensor.wait_ge(s_cpk2, 16)  # W2 + b1 consts
                        if n >= 1:
                            tensor.wait_ge(s_act, n)
                        if n >= 2:
                            tensor.wait_ge(s_cp, 2 * (n - 1))
                        xwait(tensor, n + 1, 0)
                    for s in range(2):
                        ssl = slice(s * 512, (s + 1) * 512)
                        last = nc.tensor.matmul(
                            p1[:, ssl],
                            W1[k],
                            xsrc(n, k)[:, ssl],
                            start=(k == 0),
                            stop=(k == KB - 1),
                        )
                last.then_inc(s_l1, 1)
                if n >= 1:
                    for s in range(2):
                        idx = 2 * (n - 1) + s
                        l2_mm(
                            slice(s * 512, (s + 1) * 512),
                            slice(0, 512),
                            idx % NPS2,
                            n - 1,
                        )

            # ---- chunk 7: s-major halves (PSUM-bank aligned) ----
            n = NCHUNK - 1
            p1 = ps1[n % NPS1]
            tensor.wait_ge(s_act, n - 2)  # ps1 ring (relu(4) done)
            xwait(tensor, n, 1)  # second half data (hoisted)
            for k in range(KB):
                last = nc.tensor.matmul(
                    p1[:, 0:512],
                    W1[k],
                    xsrc(n, k)[:, 0:512],
                    start=(k == 0),
                    stop=(k == KB - 1),
                )
            last.then_inc(s_l1a, 1)
            # L2(6) pair slots in here (relu(6) finished during s0)
            tensor.wait_ge(s_act, n)
            tensor.wait_ge(s_cp, 2 * (n - 1))
            for s in range(2):
                idx = 2 * (n - 1) + s
                l2_mm(
                    slice(s * 512, (s + 1) * 512),
                    slice(0, 512),
                    idx % NPS2,
                    n - 1,
                )
            for k in range(KB):
                if k == KB - 1:
                    # hoisted deps for L2(7a): relu7a + ps2[0] ring
                    tensor.wait_ge(s_a7[0], 1)
                    tensor.wait_ge(s_cp, 13)
                last = nc.tensor.matmul(
                    p1[:, 512:1024],
                    W1[k],
                    xsrc(n, k)[:, 512:1024],
                    start=(k == 0),
                    stop=(k == KB - 1),
                )
            last.then_inc(s_l1b, 1)
            # L2(7a) on cols 0..511 (relu7a ran during s1)
            l2_mm(slice(0, 512), slice(0, 512), 0, n)
            # dummy matmuls hold the p-state while relu7b finishes (ps1[2]
            # is chunk 5's tile, long consumed; result unused)
            for _ in range(2):
                nc.tensor.matmul(
                    ps1[2][:, :512], warm_st[:, :128], WARM[:, :],
                    start=True, stop=True,
                )
            tensor.wait_ge(s_a7[1], 1)
            tensor.wait_ge(s_cp, 14)  # ps2[1] freed (copy of idx 13)
            # L2(7b) on cols 512..1023
            l2_mm(slice(512, 1024), slice(0, 512), 1, n)

        @block.scalar
        def _(scalar):
            # consts split: W1[k0..k1] first so L1(0) k0 gates on a small DMA
            scalar.dma_start(
                CONST[:, :CPK_SPLIT], cpk[:, :CPK_SPLIT]
            ).then_inc(s_cpk, 16)
            scalar.dma_start(
                CONST[:, CPK_SPLIT:], cpk[:, CPK_SPLIT:]
            ).then_inc(s_cpk2, 16)
            for n in range(NCHUNK - 1):
                if n >= NH1:
                    scalar.wait_ge(s_l2, 2 * (n - NH1) + 2)
                scalar.wait_ge(s_l1, n + 1)
                nc.scalar.activation(
                    h1[n % NH1][:], ps1[n % NPS1][:], relu, bias=B1[:]
                ).then_inc(s_act, 1)
                if n >= 2:
                    # lagged output DMA for chunk n-2 (chunks 0..4)
                    scalar.wait_ge(s_cp, 2 * (n - 1))
                    scalar.dma_start(
                        outt[:, (n - 2) * NB : (n - 1) * NB],
                        ob[(n - 2) % NOB][:],
                    ).then_inc(s_os[(n - 2) % NOB], 16)
            # chunk 7 half relus (bank A then bank B of ps1[1])
            n = NCHUNK - 1
            p1 = ps1[n % NPS1]
            scalar.wait_ge(s_l2, 10)  # h1[1] free (L2(4) done)
            scalar.wait_ge(s_l1a, 1)
            nc.scalar.activation(
                h1[n % NH1][:, 0:512], p1[:, 0:512], relu, bias=B1[:]
            ).then_inc(s_a7[0], 1)
            scalar.wait_ge(s_l1b, 1)
            nc.scalar.activation(
                h1[n % NH1][:, 512:1024], p1[:, 512:1024], relu, bias=B1[:]
            ).then_inc(s_a7[1], 1)

        @block.vector
        def _(vector):
            # initialize the warm-up operand first (nonzero so the PE power
            # ramp is actually exercised); vector is idle at block start
            nc.vector.memset(WARM[:], 2.5).then_inc(s_warm, 1)
            # chunks 0..6: two 512-col copies each
            for n in range(NCHUNK - 1):
                for s in range(2):
                    idx = 2 * n + s
                    vector.wait_ge(s_l2, idx + 1)
                    if s == 0 and n >= NOB:
                        vector.wait_ge(s_os[n % NOB], 16 * (n // NOB))
                    ssl = slice(s * 512, (s + 1) * 512)
                    nc.vector.tensor_copy(
                        ob[n % NOB][:, ssl], ps2[idx % NPS2][:, 0:512]
                    ).then_inc(s_cp, 1)
            # chunk 7 halves
            n = NCHUNK - 1
            vector.wait_ge(s_os[n % NOB], 32)  # ob[1] freed (chunks 1,4 out)
            vector.wait_ge(s_l2, 15)
            nc.vector.tensor_copy(
                ob[n % NOB][:, 0:512], ps2[0][:, 0:512]
            ).then_inc(s_cp, 1)
            vector.wait_ge(s_l2, 16)
            nc.vector.tensor_copy(
                ob[n % NOB][:, 512:1024], ps2[1][:, 0:512]
            ).then_inc(s_cp, 1)

    nc.compile()
    return nc


def _get_module():
    nc = _CACHE.get("nc")
    if nc is None:
        nc = _build_module()
        _CACHE["nc"] = nc
    return nc


def _prepare_inputs(x, conv_w, w1, b1, w2, b2):
    x = np.asarray(x, dtype=np.float32)
    conv_w = np.asarray(conv_w, dtype=np.float32)
    w1 = np.asarray(w1, dtype=np.float32)
    b1 = np.asarray(b1, dtype=np.float32)
    w2 = np.asarray(w2, dtype=np.float32)
    b2 = np.asarray(b2, dtype=np.float32)

    # Fold the 3x3 cross-correlation into w1.
    w1im = w1.reshape(HID, OUT_HW, OUT_HW)
    w1_eff = np.zeros((HID, IMG, IMG), np.float32)
    for di in range(KSZ):
        for dj in range(KSZ):
            w1_eff[:, di : di + OUT_HW, dj : dj + OUT_HW] += conv_w[di, dj] * w1im

    # 1/XSCALE folded into W1 (exact: power-of-2 scale on bf16)
    w1t_pad = np.zeros((FEAT, HPAD), _BF16)
    w1t_pad[:, :HID] = (w1_eff.reshape(HID, FEAT).T / XSCALE).astype(_BF16)
    b1_pad = np.zeros(HPAD, np.float32)
    b1_pad[:HID] = b1
    b1_pad[HID] = 1.0  # h1 row 100 == relu(0+1) == 1: carries b2
    w2t_pad = np.zeros((HPAD, NCLS), _BF16)
    w2t_pad[:HID, :] = w2.T.astype(_BF16)
    w2t_pad[HID, :] = b2.astype(_BF16)

    # blocked W1: partitions 0..111 hold w1t_pad[k*112 + p, :] at col k
    w1m_host = np.zeros((128, KB * HPAD), _BF16)
    w1m_host[:KP] = np.ascontiguousarray(
        w1t_pad.reshape(KB, KP, HPAD).transpose(1, 0, 2)
    ).reshape(KP, KB * HPAD)

    cpk = np.empty((128, CPK_BYTES), np.uint8)
    cpk[:, :1792] = w1m_host.view(np.uint8)
    cpk[:, 1792:1812] = w2t_pad.view(np.uint8)
    cpk[:, 1812:1816] = b1_pad.reshape(128, 1).view(np.uint8)

    xb = (x * XSCALE).astype(_F8)
    # xm[n, p, k*NB+b] = xq[n*NB+b, k*112+p]
    xcores = xb.reshape(N_CORES, NCHUNK, NB, KB, KP)
    xm_all = np.ascontiguousarray(xcores.transpose(0, 1, 4, 3, 2)).reshape(
        N_CORES, NCHUNK, KP, XBYTES
    )

    return [{"xm": xm_all[i], "cpk": cpk} for i in range(N_CORES)]


def _ensure_accel_backend():
    # If the caller pinned JAX_PLATFORMS=cpu, the axon/neuron PJRT devices
    # are invisible and the SPMD run would fail; undo that for this process.
    import os

    import jax

    try:
        if all(d.platform == "cpu" for d in jax.devices()):
            if os.environ.get("JAX_PLATFORMS"):
                os.environ["JAX_PLATFORMS"] = ""
                from jax.extend import backend as _jeb

                _jeb.clear_backends()
    except Exception:
        pass


def _run_device(in_maps, trace=False, trace_cores=None):
    _ensure_accel_backend()
    from concourse.bass_utils import run_bass_kernel_spmd

    nc = _get_module()
    return run_bass_kernel_spmd(
        nc,
        in_maps,
        core_ids=list(range(N_CORES)),
        trace=trace,
        trace_cores=trace_cores,
    )


def kernel(x, conv_w, w1, b1, w2, b2):
    in_maps = _prepare_inputs(x, conv_w, w1, b1, w2, b2)
    res = _run_device(in_maps)
    out = np.empty((B, NCLS), np.float32)
    for i in range(N_CORES):
        out[i * BPC : (i + 1) * BPC] = res.results[i]["outt"].T
    return out


# revision 25
# speedup vs baseline: 1.2334x; 1.0092x over previous
"""Trainium2 Bass kernel for DigitConvolutionalModel (self-contained).

Model: out = relu(conv3x3(x) @ w1.T + b1) @ w2.T + b2, x: [65536, 784] f32.

The 3x3 valid cross-correlation is linear in x, so it is folded into the
first linear layer on the host, giving a 2-layer MLP:
out = relu(x @ W1_eff.T + b1) @ w2.T + b2.

Sharding: pure data parallelism - batch split 8 ways (8192 rows/core),
weights replicated; no collectives. Per core the kernel computes
out.T [10, 8192] with batch on the matmul free dim and features on
partitions. Host casts 2*x to fp8 E3M4 (scale 2 halves the subnormal
fraction; 1/2 folded into bf16 W1) and lays it out in blocked SBUF tile
order. Features are tiled 7 x 112 (784 exactly), so there is no K=16
remainder pass: per 1024-col chunk the PE runs 14 L1 matmuls + 2 L2
matmuls, all N=512 at ~216 ns (fp8e3 moving = 1 col/cycle @2.4GHz).

Schedule notes (from trace analysis):
 - PE p-state reaches 2.4 GHz only after ~8-9 us of busy time; warm-up
   matmuls run from block start and bridge into the first data with no
   idle gap (a gap resets progress and costs ~2x matmuls for a while).
 - DMA completion semaphores trail the data descriptors by 1.5-5 us when
   the queue is deep, so the stream uses half-chunk granularity and the
   PE never waits on a DMA issued less than ~2 chunks earlier; chunk 0
   streams as 4 small pieces (1+1+2+3 k-blocks) consumed k-by-k.
 - Tensor-queue semaphore waits are hoisted a couple of matmuls before
   the group boundary they guard so LDWEIGHTS prefetch is not blocked
   (kills ~100 ns per stationary switch).
 - Last chunk is processed s0[0:512] then s1[512:1024] (PSUM-bank
   aligned: the relu of one half may not touch the bank the PE is still
   writing), with relu/L2/copy/output pipelined into the PE tail; final
   output goes out as two half DMAs on the (idle by then) Sync queue.
 - hidden dim padded 100 -> 128 with zero weight columns; b1_pad[100]=1
   makes h1 row 100 == 1.0 and W2T row 100 = b2, folding the second-layer
   bias into the second matmul.
"""

import sys

import numpy as np

if "/opt/trn_rl_repo" not in sys.path:
    sys.path.insert(0, "/opt/trn_rl_repo")

import ml_dtypes

B = 65536
IMG = 28
KSZ = 3
OUT_HW = IMG - KSZ + 1  # 26
FLAT = OUT_HW * OUT_HW  # 676
HID = 100
NCLS = 10
FEAT = IMG * IMG  # 784

N_CORES = 8
BPC = B // N_CORES  # 8192 batch rows per core
KB = 7  # feature k-blocks
KP = FEAT // KB  # 112 features per block
KH0 = 4  # k-blocks in each chunk's first half DMA
HPAD = 128
NB = 1024  # batch rows per chunk
NCHUNK = BPC // NB  # 8
XBYTES = KB * NB  # 7168 bytes per partition per chunk

NXS = 8  # half-chunk x slot ring (chunks 1-7 -> 14 halves)
NPS1 = 3
NPS2 = 2
NH1 = 3
NOB = 3
NWARM = 9
CPK_BYTES = 1816
CPK_SPLIT = 1792  # all of W1 in the first consts DMA; W2+b1 in the second
C0_PIECES = ((0, 4), (4, 7))  # chunk-0 k-block pieces

_BF16 = ml_dtypes.bfloat16
_F8 = ml_dtypes.float8_e3m4
XSCALE = 2.0  # x pre-scale before fp8 (1/XSCALE folded into W1)
_CACHE = {}


def _enable_ldw_opt():
    # Rebind concourse.bass_utils.bir_verify_and_optimise with walrus's
    # --enable-ldw-opt flipped on: consecutive matmuls sharing a stationary
    # tensor reuse the loaded weights. Falls back silently if the source no
    # longer matches.
    if _CACHE.get("ldw_patched"):
        return
    _CACHE["ldw_patched"] = True
    try:
        import inspect

        import concourse.bass_utils as bu

        src = inspect.getsource(bu.bir_verify_and_optimise)
        if "--enable-ldw-opt=false" in src:
            src = src.replace("--enable-ldw-opt=false", "--enable-ldw-opt=true")
            exec(compile(src, bu.__file__, "exec"), bu.__dict__)
    except Exception:
        pass


def _build_module():
    import contextlib

    from concourse import bacc, mybir

    _enable_ldw_opt()

    nc = bacc.Bacc(
        "TRN2", target_bir_lowering=False, debug=False, num_devices=N_CORES
    )
    xm = nc.dram_tensor(
        "xm", [NCHUNK, KP, XBYTES], mybir.dt.float8e3, kind="ExternalInput"
    ).ap()
    cpk = nc.dram_tensor(
        "cpk", [128, CPK_BYTES], mybir.dt.uint8, kind="ExternalInput"
    ).ap()
    outt = nc.dram_tensor(
        "outt", [NCLS, BPC], mybir.dt.float32, kind="ExternalOutput"
    ).ap()

    relu = mybir.ActivationFunctionType.Relu
    bf = mybir.dt.bfloat16
    f8 = mybir.dt.float8e3
    f32 = mybir.dt.float32

    ctx = contextlib.ExitStack()
    with ctx:
        CONST = ctx.enter_context(
            nc.sbuf_tensor("CONST", [128, CPK_BYTES], mybir.dt.uint8)
        )
        W1 = [
            CONST[:KP, 256 * k : 256 * (k + 1)].bitcast(bf) for k in range(KB)
        ]
        W2 = CONST[:, 1792:1812].bitcast(bf)
        B1 = CONST[:, 1812:1816].bitcast(f32)
        x0 = ctx.enter_context(nc.sbuf_tensor("x0", [KP, KB, NB], f8))
        # half-chunk slots: even slots hold k0..3, odd slots k4..6
        xh = [
            ctx.enter_context(nc.sbuf_tensor(f"xh{i}", [KP, KH0, NB], f8))
            for i in range(NXS)
        ]
        h1 = [
            ctx.enter_context(nc.sbuf_tensor(f"h1_{i}", [128, NB], bf))
            for i in range(NH1)
        ]
        ob = [
            ctx.enter_context(nc.sbuf_tensor(f"ob{i}", [NCLS, NB], f32))
            for i in range(NOB)
        ]
        ps1 = [
            ctx.enter_context(nc.psum_tensor(f"ps1_{i}", [128, NB], f32))
            for i in range(NPS1)
        ]
        ps2 = [
            ctx.enter_context(nc.psum_tensor(f"ps2_{i}", [NCLS, 512], f32))
            for i in range(NPS2)
        ]
        WARM = ctx.enter_context(nc.sbuf_tensor("WARM", [128, 512], f8))

        s_cpk = ctx.enter_context(nc.semaphore("s_cpk"))
        s_cpk2 = ctx.enter_context(nc.semaphore("s_cpk2"))
        s_x0 = [
            ctx.enter_context(nc.semaphore(f"s_x0_{j}"))
            for j in range(len(C0_PIECES))
        ]
        s_xs = [ctx.enter_context(nc.semaphore(f"s_xs{i}")) for i in range(NXS)]
        s_os = [ctx.enter_context(nc.semaphore(f"s_os{i}")) for i in range(NOB)]
        s_l1 = ctx.enter_context(nc.semaphore("s_l1"))
        s_l1a = ctx.enter_context(nc.semaphore("s_l1a"))  # chunk7 s0 done
        s_l1b = ctx.enter_context(nc.semaphore("s_l1b"))  # chunk7 s1 done
        s_act = ctx.enter_context(nc.semaphore("s_act"))
        s_a7 = [ctx.enter_context(nc.semaphore(f"s_a7_{j}")) for j in range(2)]
        s_l2 = ctx.enter_context(nc.semaphore("s_l2"))
        s_cp = ctx.enter_context(nc.semaphore("s_cp"))
        s_warm = ctx.enter_context(nc.semaphore("s_warm"))

        block = ctx.enter_context(nc.Block())

        # half h of chunk n (1..7) lives in slot (2*n+h) % NXS
        xs_count = [0] * NXS
        xs_target = {}

        @block.sync
        def _(sync):
            # chunk 0 in 2 pieces; the queue stays SHALLOW until piece 0
            # completes so its semaphore does not trail the stream (the
            # laggard DMA engine effect adds 2-3 us under a deep queue)
            for j, (k0, k1) in enumerate(C0_PIECES):
                sync.dma_start(
                    x0[:, k0:k1, :],
                    xm[0, :, k0 * NB : k1 * NB].rearrange(
                        "p (c b) -> p c b", c=k1 - k0
                    ),
                ).then_inc(s_x0[j], 16)
            # chunks 1..7 as half-chunk DMAs into the slot ring
            for hh in range(2, 2 * NCHUNK):
                n, h = hh // 2, hh % 2
                slot = hh % NXS
                if hh >= NXS + 2:
                    # slot last read by chunk (hh - NXS) // 2's L1
                    sync.wait_ge(s_l1, (hh - NXS) // 2 + 1)
                k0, k1 = (0, KH0) if h == 0 else (KH0, KB)
                sync.dma_start(
                    xh[slot][:, : k1 - k0, :],
                    xm[n, :, k0 * NB : k1 * NB].rearrange(
                        "p (c b) -> p c b", c=k1 - k0
                    ),
                ).then_inc(s_xs[slot], 16)
                xs_count[slot] += 1
                xs_target[hh] = 16 * xs_count[slot]
            # chunks 5,6 outputs + chunk 7 half outputs (sync queue is idle
            # by then; scalar stays on relu cadence)
            for n in (NCHUNK - 3, NCHUNK - 2):
                sync.wait_ge(s_cp, 2 * (n + 1))
                sync.dma_start(
                    outt[:, n * NB : (n + 1) * NB], ob[n % NOB][:]
                ).then_inc(s_os[n % NOB], 16)
            base = (NCHUNK - 1) * NB
            sync.wait_ge(s_cp, 15)
            sync.dma_start(
                outt[:, base : base + 512], ob[(NCHUNK - 1) % NOB][:, :512]
            ).then_inc(s_os[(NCHUNK - 1) % NOB], 16)
            sync.wait_ge(s_cp, 16)
            sync.dma_start(
                outt[:, base + 512 : base + 1024],
                ob[(NCHUNK - 1) % NOB][:, 512:],
            ).then_inc(s_os[(NCHUNK - 1) % NOB], 16)

        def xsrc(n, k):
            if n == 0:
                return x0[:, k, :]
            h = 0 if k < KH0 else 1
            slot = (2 * n + h) % NXS
            return xh[slot][:, k - (0 if h == 0 else KH0), :]

        def xwait(tensor, n, h):
            hh = 2 * n + h
            slot = hh % NXS
            tensor.wait_ge(s_xs[slot], xs_target[hh])

        def l2_mm(hsl, psl, psi, n):
            # second-layer matmul: out.T slice <- W2T.T @ h1 slice
            nc.tensor.matmul(
                ps2[psi][:, psl],
                W2[:],
                h1[n % NH1][:, hsl],
                start=True,
                stop=True,
            ).then_inc(s_l2, 1)

        @block.tensor
        def _(tensor):
            # PE warm-up at mid p-state while the first x piece streams in.
            # Stationary is a bf16 view of the (memset) WARM tile.
            tensor.wait_ge(s_warm, 1)
            warm_st = WARM.bitcast(bf)
            for i in range(NWARM):
                nc.tensor.matmul(
                    ps1[0][:, :512],
                    warm_st[:, :128],
                    WARM[:, :],
                    start=(i == 0),
                    stop=(i == NWARM - 1),
                )
            # ---- chunks 0..6: k-major over both 512-col subtiles ----
            for n in range(NCHUNK - 1):
                p1 = ps1[n % NPS1]
                if n == 0:
                    tensor.wait_ge(s_cpk, 16)
                last = None
                for k in range(KB):
                    if n == 0:
                        for j, (k0, _k1) in enumerate(C0_PIECES):
                            if k == k0:
                                tensor.wait_ge(s_x0[j], 16)
                    elif k == KH0:
                        xwait(tensor, n, 1)
                    if k == KB - 1:
                        # hoisted deps for the L2(n-1) pair, ps2 ring and the
                        # next chunk's first x half, so the boundary matmuls
                        # and the L2 pair are wait-free (LDW prefetch works)
                        if n == 1:
                            tensor.wait_ge(s_cpk2, 16)  # W2 first use: L2(0)
                        if n >= 1:
                            tensor.wait_ge(s_act, n)
                        if n >= 2:
                            tensor.wait_ge(s_cp, 2 * (n - 1))
                        xwait(tensor, n + 1, 0)
                    for s in range(2):
                        ssl = slice(s * 512, (s + 1) * 512)
                        last = nc.tensor.matmul(
                            p1[:, ssl],
                            W1[k],
                            xsrc(n, k)[:, ssl],
                            start=(k == 0),
                            stop=(k == KB - 1),
                        )
                last.then_inc(s_l1, 1)
                if n >= 1:
                    for s in range(2):
                        idx = 2 * (n - 1) + s
                        l2_mm(
                            slice(s * 512, (s + 1) * 512),
                            slice(0, 512),
                            idx % NPS2,
                            n - 1,
                        )

            # ---- chunk 7: s-major halves (PSUM-bank aligned) ----
            n = NCHUNK - 1
            p1 = ps1[n % NPS1]
            tensor.wait_ge(s_act, n - 2)  # ps1 ring (relu(4) done)
            xwait(tensor, n, 1)  # second half data (hoisted)
            for k in range(KB):
                last = nc.tensor.matmul(
                    p1[:, 0:512],
                    W1[k],
                    xsrc(n, k)[:, 0:512],
                    start=(k == 0),
                    stop=(k == KB - 1),
                )
            last.then_inc(s_l1a, 1)
            # L2(6) pair slots in here (relu(6) finished during s0)
            tensor.wait_ge(s_act, n)
            tensor.wait_ge(s_cp, 2 * (n - 1))
            for s in range(2):
                idx = 2 * (n - 1) + s
                l2_mm(
                    slice(s * 512, (s + 1) * 512),
                    slice(0, 512),
                    idx % NPS2,
                    n - 1,
                )
            for k in range(KB):
                if k == KB - 1:
                    # hoisted deps for L2(7a): relu7a + ps2[0] ring
                    tensor.wait_ge(s_a7[0], 1)
                    tensor.wait_ge(s_cp, 13)
                last = nc.tensor.matmul(
                    p1[:, 512:1024],
                    W1[k],
                    xsrc(n, k)[:, 512:1024],
                    start=(k == 0),
                    stop=(k == KB - 1),
                )
            last.then_inc(s_l1b, 1)
            # L2(7a) on cols 0..511 (relu7a ran during s1)
            l2_mm(slice(0, 512), slice(0, 512), 0, n)
            # dummy matmuls hold the p-state while relu7b finishes (ps1[2]
            # is chunk 5's tile, long consumed; result unused)
            for _ in range(2):
                nc.tensor.matmul(
                    ps1[2][:, :512], warm_st[:, :128], WARM[:, :],
                    start=True, stop=True,
                )
            tensor.wait_ge(s_a7[1], 1)
            tensor.wait_ge(s_cp, 14)  # ps2[1] freed (copy of idx 13)
            # L2(7b) on cols 512..1023
            l2_mm(slice(512, 1024), slice(0, 512), 1, n)

        @block.scalar
        def _(scalar):
            # consts split: W1[k0..k1] first so L1(0) k0 gates on a small DMA
            scalar.dma_start(
                CONST[:, :CPK_SPLIT], cpk[:, :CPK_SPLIT]
            ).then_inc(s_cpk, 16)
            scalar.dma_start(
                CONST[:, CPK_SPLIT:], cpk[:, CPK_SPLIT:]
            ).then_inc(s_cpk2, 16)
            for n in range(NCHUNK - 1):
                if n == 0:
                    scalar.wait_ge(s_cpk2, 16)  # b1 (bias) consts
                if n >= NH1:
                    scalar.wait_ge(s_l2, 2 * (n - NH1) + 2)
                scalar.wait_ge(s_l1, n + 1)
                nc.scalar.activation(
                    h1[n % NH1][:], ps1[n % NPS1][:], relu, bias=B1[:]
                ).then_inc(s_act, 1)
                if n >= 2:
                    # lagged output DMA for chunk n-2 (chunks 0..4)
                    scalar.wait_ge(s_cp, 2 * (n - 1))
                    scalar.dma_start(
                        outt[:, (n - 2) * NB : (n - 1) * NB],
                        ob[(n - 2) % NOB][:],
                    ).then_inc(s_os[(n - 2) % NOB], 16)
            # chunk 7 half relus (bank A then bank B of ps1[1])
            n = NCHUNK - 1
            p1 = ps1[n % NPS1]
            scalar.wait_ge(s_l2, 10)  # h1[1] free (L2(4) done)
            scalar.wait_ge(s_l1a, 1)
            nc.scalar.activation(
                h1[n % NH1][:, 0:512], p1[:, 0:512], relu, bias=B1[:]
            ).then_inc(s_a7[0], 1)
            scalar.wait_ge(s_l1b, 1)
            nc.scalar.activation(
                h1[n % NH1][:, 512:1024], p1[:, 512:1024], relu, bias=B1[:]
            ).then_inc(s_a7[1], 1)

        @block.vector
        def _(vector):
            # initialize the warm-up operand first (nonzero so the PE power
            # ramp is actually exercised); vector is idle at block start
            nc.vector.memset(WARM[:], 2.5).then_inc(s_warm, 1)
            # chunks 0..6: two 512-col copies each
            for n in range(NCHUNK - 1):
                for s in range(2):
                    idx = 2 * n + s
                    vector.wait_ge(s_l2, idx + 1)
                    if s == 0 and n >= NOB:
                        vector.wait_ge(s_os[n % NOB], 16 * (n // NOB))
                    ssl = slice(s * 512, (s + 1) * 512)
                    nc.vector.tensor_copy(
                        ob[n % NOB][:, ssl], ps2[idx % NPS2][:, 0:512]
                    ).then_inc(s_cp, 1)
            # chunk 7 halves
            n = NCHUNK - 1
            vector.wait_ge(s_os[n % NOB], 32)  # ob[1] freed (chunks 1,4 out)
            vector.wait_ge(s_l2, 15)
            nc.vector.tensor_copy(
                ob[n % NOB][:, 0:512], ps2[0][:, 0:512]
            ).then_inc(s_cp, 1)
            vector.wait_ge(s_l2, 16)
            nc.vector.tensor_copy(
                ob[n % NOB][:, 512:1024], ps2[1][:, 0:512]
            ).then_inc(s_cp, 1)

    nc.compile()
    return nc


def _get_module():
    nc = _CACHE.get("nc")
    if nc is None:
        nc = _build_module()
        _CACHE["nc"] = nc
    return nc


def _prepare_inputs(x, conv_w, w1, b1, w2, b2):
    x = np.asarray(x, dtype=np.float32)
    conv_w = np.asarray(conv_w, dtype=np.float32)
    w1 = np.asarray(w1, dtype=np.float32)
    b1 = np.asarray(b1, dtype=np.float32)
    w2 = np.asarray(w2, dtype=np.float32)
    b2 = np.asarray(b2, dtype=np.float32)

    # Fold the 3x3 cross-correlation into w1.
    w1im = w1.reshape(HID, OUT_HW, OUT_HW)
    w1_eff = np.zeros((HID, IMG, IMG), np.float32)
    for di in range(KSZ):
        for dj in range(KSZ):
            w1_eff[:, di : di + OUT_HW, dj : dj + OUT_HW] += conv_w[di, dj] * w1im

    # 1/XSCALE folded into W1 (exact: power-of-2 scale on bf16)
    w1t_pad = np.zeros((FEAT, HPAD), _BF16)
    w1t_pad[:, :HID] = (w1_eff.reshape(HID, FEAT).T / XSCALE).astype(_BF16)
    b1_pad = np.zeros(HPAD, np.float32)
    b1_pad[:HID] = b1
    b1_pad[HID] = 1.0  # h1 row 100 == relu(0+1) == 1: carries b2
    w2t_pad = np.zeros((HPAD, NCLS), _BF16)
    w2t_pad[:HID, :] = w2.T.astype(_BF16)
    w2t_pad[HID, :] = b2.astype(_BF16)

    # blocked W1: partitions 0..111 hold w1t_pad[k*112 + p, :] at col k
    w1m_host = np.zeros((128, KB * HPAD), _BF16)
    w1m_host[:KP] = np.ascontiguousarray(
        w1t_pad.reshape(KB, KP, HPAD).transpose(1, 0, 2)
    ).reshape(KP, KB * HPAD)

    cpk = np.empty((128, CPK_BYTES), np.uint8)
    cpk[:, :1792] = w1m_host.view(np.uint8)
    cpk[:, 1792:1812] = w2t_pad.view(np.uint8)
    cpk[:, 1812:1816] = b1_pad.reshape(128, 1).view(np.uint8)

    xb = (x * XSCALE).astype(_F8)
    # xm[n, p, k*NB+b] = xq[n*NB+b, k*112+p]
    xcores = xb.reshape(N_CORES, NCHUNK, NB, KB, KP)
    xm_all = np.ascontiguousarray(xcores.transpose(0, 1, 4, 3, 2)).reshape(
        N_CORES, NCHUNK, KP, XBYTES
    )

    return [{"xm": xm_all[i], "cpk": cpk} for i in range(N_CORES)]


def _ensure_accel_backend():
    # If the caller pinned JAX_PLATFORMS=cpu, the axon/neuron PJRT devices
    # are invisible and the SPMD run would fail; undo that for this process.
    import os

    import jax

    try:
        if all(d.platform == "cpu" for d in jax.devices()):
            if os.environ.get("JAX_PLATFORMS"):
                os.environ["JAX_PLATFORMS"] = ""
                from jax.extend import backend as _jeb

                _jeb.clear_backends()
    except Exception:
        pass


def _run_device(in_maps, trace=False, trace_cores=None):
    _ensure_accel_backend()
    from concourse.bass_utils import run_bass_kernel_spmd

    nc = _get_module()
    return run_bass_kernel_spmd(
        nc,
        in_maps,
        core_ids=list(range(N_CORES)),
        trace=trace,
        trace_cores=trace_cores,
    )


def kernel(x, conv_w, w1, b1, w2, b2):
    in_maps = _prepare_inputs(x, conv_w, w1, b1, w2, b2)
    res = _run_device(in_maps)
    out = np.empty((B, NCLS), np.float32)
    for i in range(N_CORES):
        out[i * BPC : (i + 1) * BPC] = res.results[i]["outt"].T
    return out
